# revision 19
# baseline (speedup 1.0000x reference)
"""2-layer GCN block (GCNConv -> BN -> ReLU -> GCNConv -> BN -> +residual -> ReLU)
on 8 TRN2 NeuronCores.

Strategy (graph/data parallel, matches the sharding hint):
- Nodes are padded to 100352 = 8*12544 and sharded by contiguous range; core c
  owns rows [c*12544, (c+1)*12544) (= original nodes [c*12500,(c+1)*12500) plus
  44 pad slots). Edges are bucketed by destination owner on the host.
- GCNConv is reassociated via linearity: agg[dst] = sum_e norm_e * x[src_e]
  (self-loops become ordinary edges with norm = 1/deg), then conv = W.T @ agg
  in transposed layout [ch, node]. The bias cancels exactly through
  training-mode BatchNorm and is dropped.
- Per core, edges sorted by (dst block of 128, src bank of 25088). Source rows
  are fetched with dma_gather (int16 bank-relative indices), scattered into
  the dst block via a one-hot matmul on the TensorEngine accumulating in PSUM:
  onehot[e, d] = (iota[d] == dstmod_e) * norm_e   (one fused DVE op / tile)
  aggT[ch, d] += gathered[e, ch].T @ onehot       (one bf16 matmul / tile)
- BN stats (sum / sum-of-squares per channel) ride a tiny AllGather; the h
  shards move between layers with one 25.7MB bf16 AllGather.
- Per-core gather-group sizes are equalized across cores (pad with idx=0,
  norm=0) so all 8 cores run one identical instruction stream.
"""

import math
import os
import sys
import time

import numpy as np

for _p in ("/opt/trn_rl_repo", "/root/.axon_site/_ro/trn_rl_repo"):
    if os.path.isdir(_p) and _p not in sys.path:
        sys.path.append(_p)

import ml_dtypes

BF16_NP = ml_dtypes.bfloat16

P = 128
EPS = 1e-5


class Cfg:
    def __init__(self, n_real=100000, shard_blocks=98, ncores=8, batch_blocks=4,
                 nbanks=4, slab_bufs=2):
        self.n_real = n_real
        self.ncores = ncores
        self.blocks = shard_blocks          # 128-row blocks per core
        self.shard = shard_blocks * P       # rows per core (padded)
        self.npad = self.shard * ncores
        self.real_per_shard = n_real // ncores
        assert n_real % ncores == 0 and self.real_per_shard <= self.shard
        self.nbanks = nbanks
        assert self.npad % nbanks == 0
        self.bank_rows = self.npad // nbanks
        assert self.bank_rows <= 32767
        self.batch_blocks = batch_blocks
        self.nbatches = math.ceil(self.blocks / batch_blocks)
        self.slab_bufs = slab_bufs


CFG_FULL = Cfg()


# ----------------------------------------------------------------------------
# host-side graph preprocessing
# ----------------------------------------------------------------------------

def preprocess(edge_index, cfg: Cfg):
    """Bucket/sort/pad edges; build per-core device streams + a shared plan."""
    n, rp, sh = cfg.n_real, cfg.real_per_shard, cfg.shard
    src = np.asarray(edge_index[0], dtype=np.int64)
    dst = np.asarray(edge_index[1], dtype=np.int64)

    deg = np.bincount(dst, minlength=n).astype(np.float64) + 1.0
    dinv = 1.0 / np.sqrt(deg)
    norm = (dinv[src] * dinv[dst]).astype(np.float32)

    def to_pad(ids):
        return (ids // rp) * sh + (ids % rp)

    gsrc = to_pad(src)
    gdst = to_pad(dst)
    w = norm

    # self-loop weights per core, laid out [128, blocks] (node b*128+p), pads 0
    dinv2 = np.zeros((cfg.ncores, P, cfg.blocks), dtype=np.float32)
    d2 = (dinv * dinv).astype(np.float32)
    for c in range(cfg.ncores):
        v = np.zeros(sh, dtype=np.float32)
        v[:rp] = d2[c * rp:(c + 1) * rp]
        dinv2[c] = v.reshape(cfg.blocks, P).T

    core = gdst // sh
    dst_local = gdst - core * sh
    block = dst_local // P
    dstmod = (dst_local % P).astype(np.float32)
    bank = gsrc // cfg.bank_rows
    src_rel = (gsrc - bank * cfg.bank_rows).astype(np.int16)

    # per (core, block, bank) counts -> equalized counts
    nb = cfg.nbanks
    gkey = (core * cfg.blocks + block) * nb + bank
    counts = np.bincount(gkey, minlength=cfg.ncores * cfg.blocks * nb)
    counts = counts.reshape(cfg.ncores, cfg.blocks, nb)
    valid_eq = counts.max(axis=0)                         # [blocks, nbanks]
    slot_cnt = ((valid_eq + P - 1) // P) * P              # [blocks, nbanks]

    # ---- shared plan ----------------------------------------------------
    batches = [list(range(i, min(i + cfg.batch_blocks, cfg.blocks)))
               for i in range(0, cfg.blocks, cfg.batch_blocks)]
    plan = []
    tile_base = 0
    # group start position (in slots) inside each core stream, per (block, bank)
    grp_start = np.zeros((cfg.blocks, nb), dtype=np.int64)
    stream_pos = 0
    for bl in batches:
        calls = []
        bt0 = tile_base
        block_tiles = {b: [] for b in bl}
        for k in range(nb):
            for b in bl:
                grp_start[b, k] = stream_pos
                ntk = int(slot_cnt[b, k]) // P
                block_tiles[b].extend(range(tile_base, tile_base + ntk))
                calls.append(dict(slots=int(slot_cnt[b, k]),
                                  reg=int(valid_eq[b, k]),
                                  bank=k,
                                  tile_off=tile_base, tiles=ntk))
                tile_base += ntk
                stream_pos += int(slot_cnt[b, k])
        plan.append(dict(blocks=bl, calls=calls, tile0=bt0,
                         ntiles=tile_base - bt0,
                         block_tiles={b: block_tiles[b] for b in bl}))
    tot_tiles = tile_base
    tot_slots = tot_tiles * P

    # ---- per-core streams ----------------------------------------------
    # default fill: pads are idx 0 (valid, norm 0) except each call's trailing
    # region after the last group's equalized count, which is -1 (skipped).
    idx_flat0 = np.full(tot_slots, -1, dtype=np.int16)
    for b in range(cfg.blocks):
        for k in range(nb):
            s = grp_start[b, k]
            idx_flat0[s:s + valid_eq[b, k]] = 0
    streams = []
    for c in range(cfg.ncores):
        sel = np.nonzero(core == c)[0]
        bsel = block[sel]
        ksel = bank[sel]
        key = bsel * nb + ksel
        o = np.argsort(key, kind="stable")
        sel = sel[o]
        key = key[o]
        # rank within group
        starts = np.searchsorted(key, np.arange(cfg.blocks * nb))
        rank = np.arange(len(sel)) - starts[key]
        pos = grp_start.reshape(-1)[key] + rank

        idx_flat = idx_flat0.copy()
        nrm_flat = np.zeros(tot_slots, dtype=np.float32)
        dst_flat = np.full(tot_slots, -1.0, dtype=np.float32)
        idx_flat[pos] = src_rel[sel]
        nrm_flat[pos] = w[sel]
        dst_flat[pos] = dstmod[sel]

        # idx wrap: per call, i -> [i%16 (x8 partitions), col0 + i//16]
        idx_w = np.empty((P, tot_slots // 16), dtype=np.int16)
        for pb in plan:
            for cl in pb["calls"]:
                s0 = cl["tile_off"] * P
                ns = cl["slots"]
                wseg = idx_flat[s0:s0 + ns].reshape(ns // 16, 16).T  # [16, cols]
                idx_w[:, s0 // 16:(s0 + ns) // 16] = np.tile(wseg, (8, 1))
        nrm_w = nrm_flat.reshape(tot_tiles, P).T.copy()   # [128, tot_tiles]
        dst_w = dst_flat.reshape(tot_tiles, P).T.copy()
        streams.append(dict(idx=idx_w, nrm=nrm_w, dst=dst_w, dinv2=dinv2[c]))

    meta = dict(plan=plan, tot_tiles=tot_tiles,
                t_max=max(pb["ntiles"] for pb in plan))
    return meta, streams


# ----------------------------------------------------------------------------
# device module
# ----------------------------------------------------------------------------

def build_module(cfg: Cfg, meta):
    import concourse.bacc as bacc
    import concourse.tile as tile
    from concourse import mybir

    F32 = mybir.dt.float32
    BF16 = mybir.dt.bfloat16
    I16 = mybir.dt.int16
    AF = mybir.ActivationFunctionType
    OP = mybir.AluOpType

    plan = meta["plan"]
    tot_tiles = meta["tot_tiles"]
    t_max = meta["t_max"]
    nb = cfg.nbanks

    nc = bacc.Bacc("TRN2", target_bir_lowering=False, debug=False,
                   num_devices=cfg.ncores)

    x_gsrc = nc.dram_tensor("x_gsrc", [cfg.npad, P], BF16, kind="ExternalInput").ap()
    x_own = nc.dram_tensor("x_own", [cfg.shard, P], BF16, kind="ExternalInput").ap()
    dinv2_in = nc.dram_tensor("dinv2_s", [P, cfg.blocks], F32, kind="ExternalInput").ap()
    rowidx_in = nc.dram_tensor("rowidx", [P, 1], F32, kind="ExternalInput").ap()
    xT_id = nc.dram_tensor("xT_id", [P, cfg.shard], F32, kind="ExternalInput").ap()
    idx_in = nc.dram_tensor("idx_s", [P, tot_tiles * 8], I16, kind="ExternalInput").ap()
    nrm_in = nc.dram_tensor("nrm_s", [P, tot_tiles], F32, kind="ExternalInput").ap()
    dst_in = nc.dram_tensor("dst_s", [P, tot_tiles], F32, kind="ExternalInput").ap()
    w1_in = nc.dram_tensor("w1", [P, P], F32, kind="ExternalInput").ap()
    w2_in = nc.dram_tensor("w2", [P, P], F32, kind="ExternalInput").ap()
    gamma_in = nc.dram_tensor("gamma", [P], F32, kind="ExternalInput").ap()
    beta_in = nc.dram_tensor("beta", [P], F32, kind="ExternalInput").ap()
    iota_in = nc.dram_tensor("iota_c", [P, P], BF16, kind="ExternalInput").ap()
    idb_in = nc.dram_tensor("ident_b", [P, P], BF16, kind="ExternalInput").ap()
    idf_in = nc.dram_tensor("ident_f", [P, P], F32, kind="ExternalInput").ap()
    sel_in = nc.dram_tensor("ones_sel", [P, 2], F32, kind="ExternalInput").ap()

    out_d = nc.dram_tensor("out", [cfg.shard, P], F32, kind="ExternalOutput").ap()

    inv_n = 1.0 / float(cfg.n_real)

    with tile.TileContext(nc) as tc:
        with tc.tile_pool(name="c1", bufs=1) as c1, \
             tc.tile_pool(name="sb", bufs=2) as sb, \
             tc.tile_pool(name="ps", bufs=2, space="PSUM") as ps, \
             tc.tile_pool(name="dram", bufs=1, space="DRAM") as dram:

            # ---------------- constants / persistents
            iota_t = c1.tile([P, P], BF16)
            identb_t = c1.tile([P, P], BF16)
            identf_t = c1.tile([P, P], F32)
            sel_t = c1.tile([P, 2], F32)
            w1_t = c1.tile([P, P], F32)
            w2_t = c1.tile([P, P], F32)
            gamma_t = c1.tile([P, 1], F32)
            beta_t = c1.tile([P, 1], F32)
            rowidx_t = c1.tile([P, 1], F32)
            dinv2_t = c1.tile([P, cfg.blocks], F32)
            nc.sync.dma_start(rowidx_t[:], rowidx_in[:])
            nc.sync.dma_start(dinv2_t[:], dinv2_in[:])
            nc.sync.dma_start(iota_t[:], iota_in[:])
            nc.sync.dma_start(identb_t[:], idb_in[:])
            nc.sync.dma_start(identf_t[:], idf_in[:])
            nc.sync.dma_start(sel_t[:], sel_in[:])
            nc.sync.dma_start(w1_t[:], w1_in[:])
            nc.sync.dma_start(w2_t[:], w2_in[:])
            nc.sync.dma_start(gamma_t[:], gamma_in[:])
            nc.sync.dma_start(beta_t[:], beta_in[:])

            conv1_sb = c1.tile([P, cfg.shard], BF16)     # layer1 conv (pre-BN)
            conv2_sb = c1.tile([P, cfg.shard], F32)      # layer2 conv (pre-BN)

            ag_h_in = dram.tile([cfg.shard, P], BF16)
            ag_h_out = dram.tile([cfg.npad, P], BF16, addr_space="Shared")
            stats_in = [dram.tile([2, P], F32, name=f"stats_in{l}") for l in range(2)]
            stats_out = [dram.tile([2 * cfg.ncores, P], F32, addr_space="Shared",
                                   name=f"stats_out{l}") for l in range(2)]

            slab_allocs = [0]

            def emit_layer(lyr):
                src_ap = x_gsrc if lyr == 0 else ag_h_out
                own_ap = x_own if lyr == 0 else ag_h_in
                w_t = w1_t if lyr == 0 else w2_t
                conv_sb = conv1_sb if lyr == 0 else conv2_sb
                s_part = c1.tile([P, cfg.blocks], F32, name=f"S{lyr}")
                q_part = c1.tile([P, cfg.blocks], F32, name=f"Q{lyr}")

                for bi, pb in enumerate(plan):
                    t0 = pb["tile0"]
                    ntl = pb["ntiles"]
                    slab = sb.tile([P, t_max, P], BF16, tag="slab", bufs=cfg.slab_bufs)
                    if slab_allocs[0] < cfg.slab_bufs:
                        nc.vector.memset(slab[:], 0)
                    slab_allocs[0] += 1
                    idx_t = sb.tile([P, t_max * 8], I16, tag="idx", bufs=2)
                    nrm_t = sb.tile([P, t_max], F32, tag="nrm", bufs=2)
                    dst_t = sb.tile([P, t_max], F32, tag="dst", bufs=2)
                    nc.sync.dma_start(idx_t[:, :ntl * 8], idx_in[:, t0 * 8:(t0 + ntl) * 8])
                    nc.sync.dma_start(nrm_t[:, :ntl], nrm_in[:, t0:t0 + ntl])
                    nc.sync.dma_start(dst_t[:, :ntl], dst_in[:, t0:t0 + ntl])

                    for cl in pb["calls"]:
                        if cl["tiles"] == 0 or cl["reg"] == 0:
                            continue
                        k = cl["bank"]
                        lt0 = cl["tile_off"] - t0
                        nc.gpsimd.dma_gather(
                            slab[:, lt0:lt0 + cl["tiles"], :],
                            src_ap[k * cfg.bank_rows:(k + 1) * cfg.bank_rows, :],
                            idx_t[:, lt0 * 8:(lt0 + cl["tiles"]) * 8],
                            cl["slots"],
                            cl["reg"],
                            P,
                            elem_step=P,
                        )

                    for b in pb["blocks"]:
                        tiles = pb["block_tiles"][b]
                        agg_ps = ps.tile([P, P], F32, tag="agg", bufs=2)
                        for j, t in enumerate(tiles):
                            lt = t - t0
                            oh = sb.tile([P, P], BF16, tag="oh", bufs=4)
                            nc.vector.tensor_scalar(
                                oh[:], iota_t[:],
                                dst_t[:, lt:lt + 1], nrm_t[:, lt:lt + 1],
                                OP.is_equal, OP.mult,
                            )
                            nc.tensor.matmul(
                                out=agg_ps[:], lhsT=slab[:, lt, :], rhs=oh[:],
                                start=(j == 0), stop=False,
                            )
                        # self-loop: agg[ch, d] += x_own[d, ch] * dinv2[d]
                        oh_s = sb.tile([P, P], BF16, tag="oh", bufs=4)
                        nc.vector.tensor_scalar(
                            oh_s[:], iota_t[:],
                            rowidx_t[:], dinv2_t[:, b:b + 1],
                            OP.is_equal, OP.mult,
                        )
                        xo = sb.tile([P, P], BF16, tag="xo", bufs=3)
                        nc.sync.dma_start(xo[:], own_ap[b * P:(b + 1) * P, :])
                        nc.tensor.matmul(
                            out=agg_ps[:], lhsT=xo[:], rhs=oh_s[:],
                            start=(len(tiles) == 0), stop=True,
                        )
                        aggT = sb.tile([P, P], F32, tag="aggT", bufs=3)
                        nc.vector.tensor_copy(aggT[:], agg_ps[:])
                        cps = ps.tile([P, P], F32, tag="conv", bufs=2)
                        nc.tensor.matmul(out=cps[:], lhsT=w_t[:], rhs=aggT[:],
                                         start=True, stop=True)
                        # copy to conv store + per-channel sum via accumulator
                        nc.scalar.activation(
                            out=conv_sb[:, b * P:(b + 1) * P], in_=cps[:],
                            func=AF.Copy, accum_out=s_part[:, b:b + 1])
                        sq = sb.tile([P, P], F32, tag="sq", bufs=2)
                        nc.scalar.activation(
                            out=sq[:], in_=cps[:], func=AF.Square,
                            accum_out=q_part[:, b:b + 1])

                # ---- stats allgather + affine coefficients
                s_red = sb.tile([P, 1], F32, tag="sred", bufs=2)
                q_red = sb.tile([P, 1], F32, tag="qred", bufs=2)
                nc.vector.tensor_reduce(s_red[:], s_part[:], mybir.AxisListType.X, OP.add)
                nc.vector.tensor_reduce(q_red[:], q_part[:], mybir.AxisListType.X, OP.add)
                nc.sync.dma_start(stats_in[lyr][0], s_red[:])
                nc.sync.dma_start(stats_in[lyr][1], q_red[:])
                nc.gpsimd.collective_compute(
                    "AllGather", OP.bypass,
                    replica_groups=[list(range(cfg.ncores))],
                    ins=[stats_in[lyr].opt()],
                    outs=[stats_out[lyr].opt()],
                )
                stats_sb = sb.tile([P, P], F32, tag="stats_sb", bufs=2)
                nc.vector.memset(stats_sb[:], 0)
                nc.sync.dma_start(stats_sb[:2 * cfg.ncores, :P], stats_out[lyr][:])
                stat_ps = ps.tile([P, 2], F32, tag="stat_ps", bufs=1)
                nc.tensor.matmul(out=stat_ps[:], lhsT=stats_sb[:], rhs=sel_t[:],
                                 start=True, stop=True)
                mu = sb.tile([P, 1], F32, tag="mu", bufs=2)
                msq = sb.tile([P, 1], F32, tag="msq", bufs=2)
                var = sb.tile([P, 1], F32, tag="var", bufs=2)
                sd = sb.tile([P, 1], F32, tag="sd", bufs=2)
                rs = sb.tile([P, 1], F32, tag="rs", bufs=2)
                s_co = sb.tile([P, 1], F32, tag="s_co", bufs=2)
                t_co = sb.tile([P, 1], F32, tag="t_co", bufs=2)
                nc.vector.tensor_scalar(mu[:], stat_ps[:, 0:1], inv_n, None, OP.mult)
                nc.vector.tensor_scalar(msq[:], stat_ps[:, 1:2], inv_n, None, OP.mult)
                nc.vector.tensor_tensor(out=var[:], in0=mu[:], in1=mu[:], op=OP.mult)
                nc.vector.tensor_tensor(out=var[:], in0=msq[:], in1=var[:], op=OP.subtract)
                nc.vector.tensor_scalar(var[:], var[:], EPS, None, OP.add)
                nc.scalar.activation(out=sd[:], in_=var[:], func=AF.Sqrt)
                nc.vector.reciprocal(rs[:], sd[:])
                nc.vector.tensor_tensor(out=s_co[:], in0=gamma_t[:], in1=rs[:], op=OP.mult)
                nc.vector.tensor_tensor(out=t_co[:], in0=mu[:], in1=s_co[:], op=OP.mult)
                nc.vector.tensor_tensor(out=t_co[:], in0=beta_t[:], in1=t_co[:], op=OP.subtract)
                return s_co, t_co

            # ======== layer 1
            s1, t1 = emit_layer(0)
            for b in range(cfg.blocks):
                hT = sb.tile([P, P], BF16, tag="hT", bufs=3)
                nc.scalar.activation(out=hT[:], in_=conv1_sb[:, b * P:(b + 1) * P],
                                     func=AF.Relu, bias=t1[:], scale=s1[:])
                trp = ps.tile([P, P], BF16, tag="trb", bufs=2)
                nc.tensor.transpose(out=trp[:], in_=hT[:], identity=identb_t[:])
                hrow = sb.tile([P, P], BF16, tag="hrow", bufs=3)
                nc.vector.tensor_copy(hrow[:], trp[:])
                nc.sync.dma_start(ag_h_in[b * P:(b + 1) * P, :], hrow[:])
            nc.gpsimd.collective_compute(
                "AllGather", mybir.AluOpType.bypass,
                replica_groups=[list(range(cfg.ncores))],
                ins=[ag_h_in.opt()],
                outs=[ag_h_out.opt()],
            )

            # ======== layer 2
            s2, t2 = emit_layer(1)
            for b in range(cfg.blocks):
                bn = sb.tile([P, P], F32, tag="bn", bufs=3)
                nc.vector.tensor_scalar(bn[:], conv2_sb[:, b * P:(b + 1) * P],
                                        s2[:], t2[:], OP.mult, OP.add)
                xt = sb.tile([P, P], F32, tag="xt", bufs=3)
                nc.sync.dma_start(xt[:], xT_id[:, b * P:(b + 1) * P])
                bn2 = sb.tile([P, P], F32, tag="bn2", bufs=3)
                nc.vector.tensor_tensor(out=bn2[:], in0=bn[:], in1=xt[:], op=OP.add)
                trf = ps.tile([P, P], F32, tag="trf", bufs=1)
                nc.tensor.transpose(out=trf[:], in_=bn2[:], identity=identf_t[:])
                ot = sb.tile([P, P], F32, tag="ot", bufs=3)
                nc.scalar.activation(out=ot[:], in_=trf[:], func=AF.Relu)
                nc.sync.dma_start(out_d[b * P:(b + 1) * P, :], ot[:])

    nc.compile()
    return nc


# ----------------------------------------------------------------------------
# runner
# ----------------------------------------------------------------------------

_CACHE = {}


def _get_module(cfg: Cfg, edge_key, edge_index):
    key = ("mod", cfg.n_real, cfg.shard, edge_key)
    if key not in _CACHE:
        meta, streams = preprocess(edge_index, cfg)
        nc = build_module(cfg, meta)
        _CACHE[key] = (nc, meta, streams)
    return _CACHE[key]


def _make_in_maps(cfg: Cfg, x, W1, W2, gamma2, beta2, streams):
    n, rp, sh = cfg.n_real, cfg.real_per_shard, cfg.shard
    x = np.asarray(x, dtype=np.float32)
    x_pad = np.zeros((cfg.npad, P), dtype=np.float32)
    for c in range(cfg.ncores):
        x_pad[c * sh:c * sh + rp] = x[c * rp:(c + 1) * rp]
    x_bf = x_pad.astype(BF16_NP)

    iota = np.broadcast_to(np.arange(P, dtype=np.float32), (P, P)).astype(BF16_NP)
    identb = np.eye(P, dtype=np.float32).astype(BF16_NP)
    identf = np.eye(P, dtype=np.float32)
    sel = np.zeros((P, 2), dtype=np.float32)
    sel[0:2 * cfg.ncores:2, 0] = 1.0
    sel[1:2 * cfg.ncores:2, 1] = 1.0

    rowidx = np.arange(P, dtype=np.float32).reshape(P, 1)
    in_maps = []
    for c in range(cfg.ncores):
        xT = np.zeros((P, sh), dtype=np.float32)
        xT[:, :rp] = x[c * rp:(c + 1) * rp].T
        st = streams[c]
        in_maps.append(dict(
            x_gsrc=x_bf, x_own=np.ascontiguousarray(x_bf[c * sh:(c + 1) * sh]),
            rowidx=rowidx, dinv2_s=st["dinv2"], xT_id=xT,
            idx_s=st["idx"], nrm_s=st["nrm"], dst_s=st["dst"],
            w1=np.asarray(W1, np.float32), w2=np.asarray(W2, np.float32),
            gamma=np.asarray(gamma2, np.float32), beta=np.asarray(beta2, np.float32),
            iota_c=np.asarray(iota), ident_b=np.asarray(identb),
            ident_f=identf, ones_sel=sel,
        ))
    return in_maps


def run(x, W1, b1, W2, b2, gamma2, beta2, edge_index, cfg=CFG_FULL, trace=False):
    from concourse import bass_utils
    ei = np.asarray(edge_index)
    edge_key = hash(ei.tobytes())
    nc, meta, streams = _get_module(cfg, edge_key, ei)
    in_maps = _make_in_maps(cfg, x, W1, W2, gamma2, beta2, streams)
    res = bass_utils.run_bass_kernel_spmd(
        nc, in_maps, core_ids=list(range(cfg.ncores)), trace=trace)
    out = np.empty((cfg.n_real, P), dtype=np.float32)
    rp = cfg.real_per_shard
    for c in range(cfg.ncores):
        out[c * rp:(c + 1) * rp] = res.results[c]["out"][:rp]
    return out, res


def kernel(x, W1, b1, W2, b2, gamma2, beta2, edge_index):
    out, _ = run(x, W1, b1, W2, b2, gamma2, beta2, edge_index)
    return out


# revision 29
# speedup vs baseline: 1.0002x; 1.0002x over previous
"""2-layer GCN block (GCNConv -> BN -> ReLU -> GCNConv -> BN -> +residual -> ReLU)
on 8 TRN2 NeuronCores.

Strategy (graph/data parallel, matches the sharding hint):
- Nodes are padded to 100352 = 8*12544 and sharded by contiguous range; core c
  owns rows [c*12544, (c+1)*12544) (= original nodes [c*12500,(c+1)*12500) plus
  44 pad slots). Edges are bucketed by destination owner on the host.
- GCNConv is reassociated via linearity: agg[dst] = sum_e norm_e * x[src_e]
  (self-loops become ordinary edges with norm = 1/deg), then conv = W.T @ agg
  in transposed layout [ch, node]. The bias cancels exactly through
  training-mode BatchNorm and is dropped.
- Per core, edges sorted by (dst block of 128, src bank of 25088). Source rows
  are fetched with dma_gather (int16 bank-relative indices), scattered into
  the dst block via a one-hot matmul on the TensorEngine accumulating in PSUM:
  onehot[e, d] = (iota[d] == dstmod_e) * norm_e   (one fused DVE op / tile)
  aggT[ch, d] += gathered[e, ch].T @ onehot       (one bf16 matmul / tile)
- BN stats (sum / sum-of-squares per channel) ride a tiny AllGather; the h
  shards move between layers with one 25.7MB bf16 AllGather.
- Per-core gather-group sizes are equalized across cores (pad with idx=0,
  norm=0) so all 8 cores run one identical instruction stream.
"""

import math
import os
import sys
import time

import numpy as np

for _p in ("/opt/trn_rl_repo", "/root/.axon_site/_ro/trn_rl_repo"):
    if os.path.isdir(_p) and _p not in sys.path:
        sys.path.append(_p)

import ml_dtypes

BF16_NP = ml_dtypes.bfloat16

P = 128
EPS = 1e-5


class Cfg:
    def __init__(self, n_real=100000, shard_blocks=98, ncores=8, batch_blocks=2,
                 nbanks=4, slab_bufs=2):
        self.n_real = n_real
        self.ncores = ncores
        self.blocks = shard_blocks          # 128-row blocks per core
        self.shard = shard_blocks * P       # rows per core (padded)
        self.npad = self.shard * ncores
        self.real_per_shard = n_real // ncores
        assert n_real % ncores == 0 and self.real_per_shard <= self.shard
        self.nbanks = nbanks
        assert self.npad % nbanks == 0
        self.bank_rows = self.npad // nbanks
        assert self.bank_rows <= 32767
        self.batch_blocks = batch_blocks
        self.nbatches = math.ceil(self.blocks / batch_blocks)
        self.slab_bufs = slab_bufs


CFG_FULL = Cfg()


# ----------------------------------------------------------------------------
# host-side graph preprocessing
# ----------------------------------------------------------------------------

def preprocess(edge_index, cfg: Cfg):
    """Bucket/sort/pad edges; build per-core device streams + a shared plan."""
    n, rp, sh = cfg.n_real, cfg.real_per_shard, cfg.shard
    src = np.asarray(edge_index[0], dtype=np.int64)
    dst = np.asarray(edge_index[1], dtype=np.int64)

    deg = np.bincount(dst, minlength=n).astype(np.float64) + 1.0
    dinv = 1.0 / np.sqrt(deg)
    norm = (dinv[src] * dinv[dst]).astype(np.float32)

    def to_pad(ids):
        return (ids // rp) * sh + (ids % rp)

    gsrc = to_pad(src)
    gdst = to_pad(dst)
    w = norm

    # self-loop weights per core, laid out [128, blocks] (node b*128+p), pads 0
    dinv2 = np.zeros((cfg.ncores, P, cfg.blocks), dtype=np.float32)
    d2 = (dinv * dinv).astype(np.float32)
    for c in range(cfg.ncores):
        v = np.zeros(sh, dtype=np.float32)
        v[:rp] = d2[c * rp:(c + 1) * rp]
        dinv2[c] = v.reshape(cfg.blocks, P).T

    core = gdst // sh
    dst_local = gdst - core * sh
    block = dst_local // P
    dstmod = (dst_local % P).astype(np.float32)
    bank = gsrc // cfg.bank_rows
    src_rel = (gsrc - bank * cfg.bank_rows).astype(np.int16)

    # per (core, block, bank) counts -> equalized counts
    nb = cfg.nbanks
    gkey = (core * cfg.blocks + block) * nb + bank
    counts = np.bincount(gkey, minlength=cfg.ncores * cfg.blocks * nb)
    counts = counts.reshape(cfg.ncores, cfg.blocks, nb)
    valid_eq = counts.max(axis=0)                         # [blocks, nbanks]
    slot_cnt = ((valid_eq + P - 1) // P) * P              # [blocks, nbanks]

    # ---- shared plan ----------------------------------------------------
    batches = [list(range(i, min(i + cfg.batch_blocks, cfg.blocks)))
               for i in range(0, cfg.blocks, cfg.batch_blocks)]
    plan = []
    tile_base = 0
    # group start position (in slots) inside each core stream, per (block, bank)
    grp_start = np.zeros((cfg.blocks, nb), dtype=np.int64)
    stream_pos = 0
    for bl in batches:
        calls = []
        bt0 = tile_base
        block_tiles = {b: [] for b in bl}
        for k in range(nb):
            for b in bl:
                grp_start[b, k] = stream_pos
                ntk = int(slot_cnt[b, k]) // P
                block_tiles[b].extend(range(tile_base, tile_base + ntk))
                calls.append(dict(slots=int(slot_cnt[b, k]),
                                  reg=int(valid_eq[b, k]),
                                  bank=k, tile_off=tile_base, tiles=ntk))
                tile_base += ntk
                stream_pos += int(slot_cnt[b, k])
        plan.append(dict(blocks=bl, calls=calls, tile0=bt0,
                         ntiles=tile_base - bt0,
                         block_tiles={b: block_tiles[b] for b in bl}))
    tot_tiles = tile_base
    tot_slots = tot_tiles * P

    # ---- per-core streams ----------------------------------------------
    # default fill: pads are idx 0 (valid, norm 0) except each call's trailing
    # region after the last group's equalized count, which is -1 (skipped).
    idx_flat0 = np.full(tot_slots, -1, dtype=np.int16)
    for b in range(cfg.blocks):
        for k in range(nb):
            s = grp_start[b, k]
            idx_flat0[s:s + valid_eq[b, k]] = 0
    streams = []
    for c in range(cfg.ncores):
        sel = np.nonzero(core == c)[0]
        bsel = block[sel]
        ksel = bank[sel]
        key = bsel * nb + ksel
        o = np.argsort(key, kind="stable")
        sel = sel[o]
        key = key[o]
        # rank within group
        starts = np.searchsorted(key, np.arange(cfg.blocks * nb))
        rank = np.arange(len(sel)) - starts[key]
        pos = grp_start.reshape(-1)[key] + rank

        idx_flat = idx_flat0.copy()
        nrm_flat = np.zeros(tot_slots, dtype=np.float32)
        dst_flat = np.full(tot_slots, -1.0, dtype=np.float32)
        idx_flat[pos] = src_rel[sel]
        nrm_flat[pos] = w[sel]
        dst_flat[pos] = dstmod[sel]

        # idx wrap: per call, i -> [i%16 (x8 partitions), col0 + i//16]
        idx_w = np.empty((P, tot_slots // 16), dtype=np.int16)
        for pb in plan:
            for cl in pb["calls"]:
                s0 = cl["tile_off"] * P
                ns = cl["slots"]
                if ns == 0:
                    continue
                wseg = idx_flat[s0:s0 + ns].reshape(ns // 16, 16).T  # [16, cols]
                idx_w[:, s0 // 16:(s0 + ns) // 16] = np.tile(wseg, (8, 1))
        # host-built one-hot scatter matrices, bf16: slot i -> row (i), col dst
        oh_flat = np.zeros((tot_slots, P), dtype=np.float32)
        vmask = dst_flat >= 0
        oh_flat[np.nonzero(vmask)[0], dst_flat[vmask].astype(np.int64)] = nrm_flat[vmask]
        oh_w = np.ascontiguousarray(
            oh_flat.reshape(tot_tiles, P, P).transpose(1, 0, 2)
                   .reshape(P, tot_tiles * P)).astype(BF16_NP)
        streams.append(dict(idx=idx_w, oh=oh_w, dinv2=dinv2[c]))

    meta = dict(plan=plan, tot_tiles=tot_tiles,
                t_max=max(pb["ntiles"] for pb in plan))
    return meta, streams


# ----------------------------------------------------------------------------
# device module
# ----------------------------------------------------------------------------

def build_module(cfg: Cfg, meta):
    import concourse.bacc as bacc
    import concourse.tile as tile
    from concourse import mybir

    F32 = mybir.dt.float32
    BF16 = mybir.dt.bfloat16
    I16 = mybir.dt.int16
    AF = mybir.ActivationFunctionType
    OP = mybir.AluOpType

    plan = meta["plan"]
    tot_tiles = meta["tot_tiles"]
    t_max = meta["t_max"]
    nb = cfg.nbanks

    nc = bacc.Bacc("TRN2", target_bir_lowering=False, debug=False,
                   num_devices=cfg.ncores)

    x_gsrc = nc.dram_tensor("x_gsrc", [cfg.npad, P], BF16, kind="ExternalInput").ap()
    x_own = nc.dram_tensor("x_own", [cfg.shard, P], BF16, kind="ExternalInput").ap()
    dinv2_in = nc.dram_tensor("dinv2_s", [P, cfg.blocks], F32, kind="ExternalInput").ap()
    rowidx_in = nc.dram_tensor("rowidx", [P, 1], F32, kind="ExternalInput").ap()
    xT_id = nc.dram_tensor("xT_id", [P, cfg.shard], F32, kind="ExternalInput").ap()
    idx_in = nc.dram_tensor("idx_s", [P, tot_tiles * 8], I16, kind="ExternalInput").ap()
    oh_in = nc.dram_tensor("oh_s", [P, tot_tiles * P], BF16, kind="ExternalInput").ap()
    w1_in = nc.dram_tensor("w1", [P, P], F32, kind="ExternalInput").ap()
    w2_in = nc.dram_tensor("w2", [P, P], F32, kind="ExternalInput").ap()
    gamma_in = nc.dram_tensor("gamma", [P], F32, kind="ExternalInput").ap()
    beta_in = nc.dram_tensor("beta", [P], F32, kind="ExternalInput").ap()
    iota_in = nc.dram_tensor("iota_c", [P, P], BF16, kind="ExternalInput").ap()
    idb_in = nc.dram_tensor("ident_b", [P, P], BF16, kind="ExternalInput").ap()
    idf_in = nc.dram_tensor("ident_f", [P, P], F32, kind="ExternalInput").ap()
    sel_in = nc.dram_tensor("ones_sel", [P, 2], F32, kind="ExternalInput").ap()

    out_d = nc.dram_tensor("out", [cfg.shard, P], F32, kind="ExternalOutput").ap()

    inv_n = 1.0 / float(cfg.n_real)

    with tile.TileContext(nc) as tc:
        with tc.tile_pool(name="c1", bufs=1) as c1, \
             tc.tile_pool(name="sb", bufs=2) as sb, \
             tc.tile_pool(name="ps", bufs=2, space="PSUM") as ps, \
             tc.tile_pool(name="dram", bufs=1, space="DRAM") as dram:

            # ---------------- constants / persistents
            iota_t = c1.tile([P, P], BF16)
            identb_t = c1.tile([P, P], BF16)
            identf_t = c1.tile([P, P], F32)
            sel_t = c1.tile([P, 2], F32)
            w1_t = c1.tile([P, P], F32)
            w2_t = c1.tile([P, P], F32)
            gamma_t = c1.tile([P, 1], F32)
            beta_t = c1.tile([P, 1], F32)
            rowidx_t = c1.tile([P, 1], F32)
            dinv2_t = c1.tile([P, cfg.blocks], F32)
            nc.sync.dma_start(rowidx_t[:], rowidx_in[:])
            nc.sync.dma_start(dinv2_t[:], dinv2_in[:])
            nc.sync.dma_start(iota_t[:], iota_in[:])
            nc.sync.dma_start(identb_t[:], idb_in[:])
            nc.sync.dma_start(identf_t[:], idf_in[:])
            nc.sync.dma_start(sel_t[:], sel_in[:])
            nc.sync.dma_start(w1_t[:], w1_in[:])
            nc.sync.dma_start(w2_t[:], w2_in[:])
            nc.sync.dma_start(gamma_t[:], gamma_in[:])
            nc.sync.dma_start(beta_t[:], beta_in[:])

            conv1_sb = c1.tile([P, cfg.shard], BF16)     # layer1 conv (pre-BN)
            conv2_sb = c1.tile([P, cfg.shard], F32)      # layer2 conv (pre-BN)

            ag_h_in = dram.tile([cfg.shard, P], BF16)
            ag_h_out = dram.tile([cfg.npad, P], BF16, addr_space="Shared")
            stats_in = [dram.tile([2, P], F32, name=f"stats_in{l}") for l in range(2)]
            stats_out = [dram.tile([2 * cfg.ncores, P], F32, addr_space="Shared",
                                   name=f"stats_out{l}") for l in range(2)]

            slab_allocs = [0]

            def emit_layer(lyr):
                src_ap = x_gsrc if lyr == 0 else ag_h_out
                own_ap = x_own if lyr == 0 else ag_h_in
                w_t = w1_t if lyr == 0 else w2_t
                conv_sb = conv1_sb if lyr == 0 else conv2_sb
                s_part = c1.tile([P, cfg.blocks], F32, name=f"S{lyr}")
                q_part = c1.tile([P, cfg.blocks], F32, name=f"Q{lyr}")

                for bi, pb in enumerate(plan):
                    t0 = pb["tile0"]
                    ntl = pb["ntiles"]
                    slab = sb.tile([P, t_max, P], BF16, tag="slab", bufs=cfg.slab_bufs)
                    if slab_allocs[0] < cfg.slab_bufs:
                        nc.vector.memset(slab[:], 0)
                    slab_allocs[0] += 1
                    idx_t = sb.tile([P, t_max * 8], I16, tag="idx", bufs=2)
                    oh_t = sb.tile([P, t_max * P], BF16, tag="oh_s", bufs=2)
                    nc.sync.dma_start(idx_t[:, :ntl * 8], idx_in[:, t0 * 8:(t0 + ntl) * 8])
                    nc.sync.dma_start(oh_t[:, :ntl * P], oh_in[:, t0 * P:(t0 + ntl) * P])

                    for cl in pb["calls"]:
                        if cl["tiles"] == 0 or cl["reg"] == 0:
                            continue
                        k = cl["bank"]
                        lt0 = cl["tile_off"] - t0
                        nc.gpsimd.dma_gather(
                            slab[:, lt0:lt0 + cl["tiles"], :],
                            src_ap[k * cfg.bank_rows:(k + 1) * cfg.bank_rows, :],
                            idx_t[:, lt0 * 8:(lt0 + cl["tiles"]) * 8],
                            cl["slots"],
                            cl["reg"],
                            P,
                            elem_step=P,
                        )

                    for b in pb["blocks"]:
                        tiles = pb["block_tiles"][b]
                        agg_ps = ps.tile([P, P], F32, tag="agg", bufs=2)
                        for j, t in enumerate(tiles):
                            lt = t - t0
                            nc.tensor.matmul(
                                out=agg_ps[:], lhsT=slab[:, lt, :],
                                rhs=oh_t[:, lt * P:(lt + 1) * P],
                                start=(j == 0), stop=False,
                            )
                        # self-loop: agg[ch, d] += x_own[d, ch] * dinv2[d]
                        oh_s = sb.tile([P, P], BF16, tag="oh", bufs=4)
                        nc.vector.tensor_scalar(
                            oh_s[:], iota_t[:],
                            rowidx_t[:], dinv2_t[:, b:b + 1],
                            OP.is_equal, OP.mult,
                        )
                        xo = sb.tile([P, P], BF16, tag="xo", bufs=3)
                        nc.sync.dma_start(xo[:], own_ap[b * P:(b + 1) * P, :])
                        nc.tensor.matmul(
                            out=agg_ps[:], lhsT=xo[:], rhs=oh_s[:],
                            start=(len(tiles) == 0), stop=True,
                        )
                        aggT = sb.tile([P, P], F32, tag="aggT", bufs=3)
                        nc.vector.tensor_copy(aggT[:], agg_ps[:])
                        cps = ps.tile([P, P], F32, tag="conv", bufs=2)
                        nc.tensor.matmul(out=cps[:], lhsT=w_t[:], rhs=aggT[:],
                                         start=True, stop=True)
                        # copy to conv store + per-channel sum via accumulator
                        nc.scalar.activation(
                            out=conv_sb[:, b * P:(b + 1) * P], in_=cps[:],
                            func=AF.Copy, accum_out=s_part[:, b:b + 1])
                        sq = sb.tile([P, P], F32, tag="sq", bufs=2)
                        nc.scalar.activation(
                            out=sq[:], in_=cps[:], func=AF.Square,
                            accum_out=q_part[:, b:b + 1])

                # ---- stats allgather + affine coefficients
                s_red = sb.tile([P, 1], F32, tag="sred", bufs=2)
                q_red = sb.tile([P, 1], F32, tag="qred", bufs=2)
                nc.vector.tensor_reduce(s_red[:], s_part[:], mybir.AxisListType.X, OP.add)
                nc.vector.tensor_reduce(q_red[:], q_part[:], mybir.AxisListType.X, OP.add)
                nc.sync.dma_start(stats_in[lyr][0], s_red[:])
                nc.sync.dma_start(stats_in[lyr][1], q_red[:])
                nc.gpsimd.collective_compute(
                    "AllGather", OP.bypass,
                    replica_groups=[list(range(cfg.ncores))],
                    ins=[stats_in[lyr].opt()],
                    outs=[stats_out[lyr].opt()],
                )
                stats_sb = sb.tile([P, P], F32, tag="stats_sb", bufs=2)
                nc.vector.memset(stats_sb[:], 0)
                nc.sync.dma_start(stats_sb[:2 * cfg.ncores, :P], stats_out[lyr][:])
                stat_ps = ps.tile([P, 2], F32, tag="stat_ps", bufs=1)
                nc.tensor.matmul(out=stat_ps[:], lhsT=stats_sb[:], rhs=sel_t[:],
                                 start=True, stop=True)
                mu = sb.tile([P, 1], F32, tag="mu", bufs=2)
                msq = sb.tile([P, 1], F32, tag="msq", bufs=2)
                var = sb.tile([P, 1], F32, tag="var", bufs=2)
                sd = sb.tile([P, 1], F32, tag="sd", bufs=2)
                rs = sb.tile([P, 1], F32, tag="rs", bufs=2)
                s_co = sb.tile([P, 1], F32, tag="s_co", bufs=2)
                t_co = sb.tile([P, 1], F32, tag="t_co", bufs=2)
                nc.vector.tensor_scalar(mu[:], stat_ps[:, 0:1], inv_n, None, OP.mult)
                nc.vector.tensor_scalar(msq[:], stat_ps[:, 1:2], inv_n, None, OP.mult)
                nc.vector.tensor_tensor(out=var[:], in0=mu[:], in1=mu[:], op=OP.mult)
                nc.vector.tensor_tensor(out=var[:], in0=msq[:], in1=var[:], op=OP.subtract)
                nc.vector.tensor_scalar(var[:], var[:], EPS, None, OP.add)
                nc.scalar.activation(out=sd[:], in_=var[:], func=AF.Sqrt)
                nc.vector.reciprocal(rs[:], sd[:])
                nc.vector.tensor_tensor(out=s_co[:], in0=gamma_t[:], in1=rs[:], op=OP.mult)
                nc.vector.tensor_tensor(out=t_co[:], in0=mu[:], in1=s_co[:], op=OP.mult)
                nc.vector.tensor_tensor(out=t_co[:], in0=beta_t[:], in1=t_co[:], op=OP.subtract)
                return s_co, t_co

            # ======== layer 1
            s1, t1 = emit_layer(0)
            for b in range(cfg.blocks):
                hT = sb.tile([P, P], BF16, tag="hT", bufs=3)
                nc.scalar.activation(out=hT[:], in_=conv1_sb[:, b * P:(b + 1) * P],
                                     func=AF.Relu, bias=t1[:], scale=s1[:])
                trp = ps.tile([P, P], BF16, tag="trb", bufs=2)
                nc.tensor.transpose(out=trp[:], in_=hT[:], identity=identb_t[:])
                hrow = sb.tile([P, P], BF16, tag="hrow", bufs=3)
                nc.vector.tensor_copy(hrow[:], trp[:])
                nc.sync.dma_start(ag_h_in[b * P:(b + 1) * P, :], hrow[:])
            nc.gpsimd.collective_compute(
                "AllGather", mybir.AluOpType.bypass,
                replica_groups=[list(range(cfg.ncores))],
                ins=[ag_h_in.opt()],
                outs=[ag_h_out.opt()],
            )

            # ======== layer 2
            s2, t2 = emit_layer(1)
            for b in range(cfg.blocks):
                bn = sb.tile([P, P], F32, tag="bn", bufs=3)
                nc.vector.tensor_scalar(bn[:], conv2_sb[:, b * P:(b + 1) * P],
                                        s2[:], t2[:], OP.mult, OP.add)
                xt = sb.tile([P, P], F32, tag="xt", bufs=3)
                nc.sync.dma_start(xt[:], xT_id[:, b * P:(b + 1) * P])
                bn2 = sb.tile([P, P], F32, tag="bn2", bufs=3)
                nc.vector.tensor_tensor(out=bn2[:], in0=bn[:], in1=xt[:], op=OP.add)
                trf = ps.tile([P, P], F32, tag="trf", bufs=1)
                nc.tensor.transpose(out=trf[:], in_=bn2[:], identity=identf_t[:])
                ot = sb.tile([P, P], F32, tag="ot", bufs=3)
                nc.scalar.activation(out=ot[:], in_=trf[:], func=AF.Relu)
                nc.sync.dma_start(out_d[b * P:(b + 1) * P, :], ot[:])

    nc.compile()
    return nc


# ----------------------------------------------------------------------------
# runner
# ----------------------------------------------------------------------------

_CACHE = {}


def _get_module(cfg: Cfg, edge_key, edge_index):
    key = ("mod", cfg.n_real, cfg.shard, edge_key)
    if key not in _CACHE:
        meta, streams = preprocess(edge_index, cfg)
        nc = build_module(cfg, meta)
        _CACHE[key] = (nc, meta, streams)
    return _CACHE[key]


def _make_in_maps(cfg: Cfg, x, W1, W2, gamma2, beta2, streams):
    n, rp, sh = cfg.n_real, cfg.real_per_shard, cfg.shard
    x = np.asarray(x, dtype=np.float32)
    x_pad = np.zeros((cfg.npad, P), dtype=np.float32)
    for c in range(cfg.ncores):
        x_pad[c * sh:c * sh + rp] = x[c * rp:(c + 1) * rp]
    x_bf = x_pad.astype(BF16_NP)

    iota = np.broadcast_to(np.arange(P, dtype=np.float32), (P, P)).astype(BF16_NP)
    identb = np.eye(P, dtype=np.float32).astype(BF16_NP)
    identf = np.eye(P, dtype=np.float32)
    sel = np.zeros((P, 2), dtype=np.float32)
    sel[0:2 * cfg.ncores:2, 0] = 1.0
    sel[1:2 * cfg.ncores:2, 1] = 1.0

    rowidx = np.arange(P, dtype=np.float32).reshape(P, 1)
    in_maps = []
    for c in range(cfg.ncores):
        xT = np.zeros((P, sh), dtype=np.float32)
        xT[:, :rp] = x[c * rp:(c + 1) * rp].T
        st = streams[c]
        in_maps.append(dict(
            x_gsrc=x_bf, x_own=np.ascontiguousarray(x_bf[c * sh:(c + 1) * sh]),
            rowidx=rowidx, dinv2_s=st["dinv2"], xT_id=xT,
            idx_s=st["idx"], oh_s=st["oh"],
            w1=np.asarray(W1, np.float32), w2=np.asarray(W2, np.float32),
            gamma=np.asarray(gamma2, np.float32), beta=np.asarray(beta2, np.float32),
            iota_c=np.asarray(iota), ident_b=np.asarray(identb),
            ident_f=identf, ones_sel=sel,
        ))
    return in_maps


def run(x, W1, b1, W2, b2, gamma2, beta2, edge_index, cfg=CFG_FULL, trace=False):
    from concourse import bass_utils
    ei = np.asarray(edge_index)
    edge_key = hash(ei.tobytes())
    nc, meta, streams = _get_module(cfg, edge_key, ei)
    in_maps = _make_in_maps(cfg, x, W1, W2, gamma2, beta2, streams)
    res = bass_utils.run_bass_kernel_spmd(
        nc, in_maps, core_ids=list(range(cfg.ncores)), trace=trace)
    out = np.empty((cfg.n_real, P), dtype=np.float32)
    rp = cfg.real_per_shard
    for c in range(cfg.ncores):
        out[c * rp:(c + 1) * rp] = res.results[c]["out"][:rp]
    return out, res


def kernel(x, W1, b1, W2, b2, gamma2, beta2, edge_index):
    out, _ = run(x, W1, b1, W2, b2, gamma2, beta2, edge_index)
    return out


# revision 31
# speedup vs baseline: 1.0680x; 1.0678x over previous
"""2-layer GCN block (GCNConv -> BN -> ReLU -> GCNConv -> BN -> +residual -> ReLU)
on 8 TRN2 NeuronCores.

Strategy (graph/data parallel, matches the sharding hint):
- Nodes are padded to 100352 = 8*12544 and sharded by contiguous range; core c
  owns rows [c*12544, (c+1)*12544) (= original nodes [c*12500,(c+1)*12500) plus
  44 pad slots). Edges are bucketed by destination owner on the host.
- GCNConv is reassociated via linearity: agg[dst] = sum_e norm_e * x[src_e]
  (self-loops become ordinary edges with norm = 1/deg), then conv = W.T @ agg
  in transposed layout [ch, node]. The bias cancels exactly through
  training-mode BatchNorm and is dropped.
- Per core, edges sorted by (dst block of 128, src bank of 25088). Source rows
  are fetched with dma_gather (int16 bank-relative indices), scattered into
  the dst block via a one-hot matmul on the TensorEngine accumulating in PSUM:
  onehot[e, d] = (iota[d] == dstmod_e) * norm_e   (one fused DVE op / tile)
  aggT[ch, d] += gathered[e, ch].T @ onehot       (one bf16 matmul / tile)
- BN stats (sum / sum-of-squares per channel) ride a tiny AllGather; the h
  shards move between layers with one 25.7MB bf16 AllGather.
- Per-core gather-group sizes are equalized across cores (pad with idx=0,
  norm=0) so all 8 cores run one identical instruction stream.
"""

import math
import os
import sys
import time

import numpy as np

for _p in ("/opt/trn_rl_repo", "/root/.axon_site/_ro/trn_rl_repo"):
    if os.path.isdir(_p) and _p not in sys.path:
        sys.path.append(_p)

import ml_dtypes

BF16_NP = ml_dtypes.bfloat16

P = 128
EPS = 1e-5


class Cfg:
    def __init__(self, n_real=100000, shard_blocks=98, ncores=8, batch_blocks=2,
                 nbanks=4, slab_bufs=2):
        self.n_real = n_real
        self.ncores = ncores
        self.blocks = shard_blocks          # 128-row blocks per core
        self.shard = shard_blocks * P       # rows per core (padded)
        self.npad = self.shard * ncores
        self.real_per_shard = n_real // ncores
        assert n_real % ncores == 0 and self.real_per_shard <= self.shard
        self.nbanks = nbanks
        assert self.npad % nbanks == 0
        self.bank_rows = self.npad // nbanks
        assert self.bank_rows <= 32767
        self.batch_blocks = batch_blocks
        self.nbatches = math.ceil(self.blocks / batch_blocks)
        self.slab_bufs = slab_bufs


CFG_FULL = Cfg()


# ----------------------------------------------------------------------------
# host-side graph preprocessing
# ----------------------------------------------------------------------------

def preprocess(edge_index, cfg: Cfg):
    """Bucket/sort/pad edges; build per-core device streams + a shared plan."""
    n, rp, sh = cfg.n_real, cfg.real_per_shard, cfg.shard
    src = np.asarray(edge_index[0], dtype=np.int64)
    dst = np.asarray(edge_index[1], dtype=np.int64)

    deg = np.bincount(dst, minlength=n).astype(np.float64) + 1.0
    dinv = 1.0 / np.sqrt(deg)
    norm = (dinv[src] * dinv[dst]).astype(np.float32)

    def to_pad(ids):
        return (ids // rp) * sh + (ids % rp)

    gsrc = to_pad(src)
    gdst = to_pad(dst)
    w = norm

    # self-loop weights per core, laid out [128, blocks] (node b*128+p), pads 0
    dinv2 = np.zeros((cfg.ncores, P, cfg.blocks), dtype=np.float32)
    d2 = (dinv * dinv).astype(np.float32)
    for c in range(cfg.ncores):
        v = np.zeros(sh, dtype=np.float32)
        v[:rp] = d2[c * rp:(c + 1) * rp]
        dinv2[c] = v.reshape(cfg.blocks, P).T

    core = gdst // sh
    dst_local = gdst - core * sh
    block = dst_local // P
    dstmod = (dst_local % P).astype(np.float32)
    bank = gsrc // cfg.bank_rows
    src_rel = (gsrc - bank * cfg.bank_rows).astype(np.int16)

    # per (core, block, bank) counts -> equalized counts
    nb = cfg.nbanks
    gkey = (core * cfg.blocks + block) * nb + bank
    counts = np.bincount(gkey, minlength=cfg.ncores * cfg.blocks * nb)
    counts = counts.reshape(cfg.ncores, cfg.blocks, nb)
    valid_eq = counts.max(axis=0)                         # [blocks, nbanks]
    slot_cnt = ((valid_eq + P - 1) // P) * P              # [blocks, nbanks]

    # ---- shared plan ----------------------------------------------------
    batches = [list(range(i, min(i + cfg.batch_blocks, cfg.blocks)))
               for i in range(0, cfg.blocks, cfg.batch_blocks)]
    plan = []
    tile_base = 0
    # group start position (in slots) inside each core stream, per (block, bank)
    grp_start = np.zeros((cfg.blocks, nb), dtype=np.int64)
    stream_pos = 0
    for bl in batches:
        calls = []
        bt0 = tile_base
        block_tiles = {b: [] for b in bl}
        wtiles = 8  # max gather-call size: 8 tiles = 1024 indices
        for k in range(nb):
            seg_t0 = tile_base
            seg_slots = 0
            seg_valid = 0
            for b in bl:
                grp_start[b, k] = stream_pos
                ntk = int(slot_cnt[b, k]) // P
                block_tiles[b].extend(range(tile_base, tile_base + ntk))
                seg_valid += int(valid_eq[b, k]) if b == bl[-1] else int(slot_cnt[b, k])
                seg_slots += int(slot_cnt[b, k])
                tile_base += ntk
                stream_pos += int(slot_cnt[b, k])
            # split this (batch, bank) segment into windows of <= wtiles tiles
            for w0 in range(0, seg_slots // P, wtiles):
                wt = min(wtiles, seg_slots // P - w0)
                wvalid = max(0, min(seg_valid - w0 * P, wt * P))
                if wvalid == 0:
                    continue
                calls.append(dict(slots=wt * P, reg=int(wvalid), bank=k,
                                  tile_off=seg_t0 + w0, tiles=wt))
        plan.append(dict(blocks=bl, calls=calls, tile0=bt0,
                         ntiles=tile_base - bt0,
                         block_tiles={b: block_tiles[b] for b in bl}))
    tot_tiles = tile_base
    tot_slots = tot_tiles * P

    # ---- per-core streams ----------------------------------------------
    # default fill: pads are idx 0 (valid, norm 0) except each call's trailing
    # region after the last group's equalized count, which is -1 (skipped).
    idx_flat0 = np.full(tot_slots, -1, dtype=np.int16)
    for bl in batches:
        for k in range(nb):
            for b in bl:
                s = grp_start[b, k]
                fill = valid_eq[b, k] if b == bl[-1] else slot_cnt[b, k]
                idx_flat0[s:s + fill] = 0
    streams = []
    for c in range(cfg.ncores):
        sel = np.nonzero(core == c)[0]
        bsel = block[sel]
        ksel = bank[sel]
        key = bsel * nb + ksel
        o = np.argsort(key, kind="stable")
        sel = sel[o]
        key = key[o]
        # rank within group
        starts = np.searchsorted(key, np.arange(cfg.blocks * nb))
        rank = np.arange(len(sel)) - starts[key]
        pos = grp_start.reshape(-1)[key] + rank

        idx_flat = idx_flat0.copy()
        nrm_flat = np.zeros(tot_slots, dtype=np.float32)
        dst_flat = np.full(tot_slots, -1.0, dtype=np.float32)
        idx_flat[pos] = src_rel[sel]
        nrm_flat[pos] = w[sel]
        dst_flat[pos] = dstmod[sel]

        # idx wrap: per call, i -> [i%16 (x8 partitions), col0 + i//16]
        idx_w = np.empty((P, tot_slots // 16), dtype=np.int16)
        for pb in plan:
            for cl in pb["calls"]:
                s0 = cl["tile_off"] * P
                ns = cl["slots"]
                if ns == 0:
                    continue
                wseg = idx_flat[s0:s0 + ns].reshape(ns // 16, 16).T  # [16, cols]
                idx_w[:, s0 // 16:(s0 + ns) // 16] = np.tile(wseg, (8, 1))
        # host-built one-hot scatter matrices, bf16: slot i -> row (i), col dst
        oh_flat = np.zeros((tot_slots, P), dtype=np.float32)
        vmask = dst_flat >= 0
        oh_flat[np.nonzero(vmask)[0], dst_flat[vmask].astype(np.int64)] = nrm_flat[vmask]
        oh_w = np.ascontiguousarray(
            oh_flat.reshape(tot_tiles, P, P).transpose(1, 0, 2)
                   .reshape(P, tot_tiles * P)).astype(BF16_NP)
        streams.append(dict(idx=idx_w, oh=oh_w, dinv2=dinv2[c]))

    meta = dict(plan=plan, tot_tiles=tot_tiles,
                t_max=max(pb["ntiles"] for pb in plan))
    return meta, streams


# ----------------------------------------------------------------------------
# device module
# ----------------------------------------------------------------------------

def build_module(cfg: Cfg, meta):
    import concourse.bacc as bacc
    import concourse.tile as tile
    from concourse import mybir

    F32 = mybir.dt.float32
    BF16 = mybir.dt.bfloat16
    I16 = mybir.dt.int16
    AF = mybir.ActivationFunctionType
    OP = mybir.AluOpType

    plan = meta["plan"]
    tot_tiles = meta["tot_tiles"]
    t_max = meta["t_max"]
    nb = cfg.nbanks

    nc = bacc.Bacc("TRN2", target_bir_lowering=False, debug=False,
                   num_devices=cfg.ncores)

    x_gsrc = nc.dram_tensor("x_gsrc", [cfg.npad, P], BF16, kind="ExternalInput").ap()
    x_own = nc.dram_tensor("x_own", [cfg.shard, P], BF16, kind="ExternalInput").ap()
    dinv2_in = nc.dram_tensor("dinv2_s", [P, cfg.blocks], F32, kind="ExternalInput").ap()
    rowidx_in = nc.dram_tensor("rowidx", [P, 1], F32, kind="ExternalInput").ap()
    xT_id = nc.dram_tensor("xT_id", [P, cfg.shard], F32, kind="ExternalInput").ap()
    idx_in = nc.dram_tensor("idx_s", [P, tot_tiles * 8], I16, kind="ExternalInput").ap()
    oh_in = nc.dram_tensor("oh_s", [P, tot_tiles * P], BF16, kind="ExternalInput").ap()
    w1_in = nc.dram_tensor("w1", [P, P], F32, kind="ExternalInput").ap()
    w2_in = nc.dram_tensor("w2", [P, P], F32, kind="ExternalInput").ap()
    gamma_in = nc.dram_tensor("gamma", [P], F32, kind="ExternalInput").ap()
    beta_in = nc.dram_tensor("beta", [P], F32, kind="ExternalInput").ap()
    iota_in = nc.dram_tensor("iota_c", [P, P], BF16, kind="ExternalInput").ap()
    idb_in = nc.dram_tensor("ident_b", [P, P], BF16, kind="ExternalInput").ap()
    idf_in = nc.dram_tensor("ident_f", [P, P], F32, kind="ExternalInput").ap()
    sel_in = nc.dram_tensor("ones_sel", [P, 2], F32, kind="ExternalInput").ap()

    out_d = nc.dram_tensor("out", [cfg.shard, P], F32, kind="ExternalOutput").ap()

    inv_n = 1.0 / float(cfg.n_real)

    with tile.TileContext(nc) as tc:
        with tc.tile_pool(name="c1", bufs=1) as c1, \
             tc.tile_pool(name="sb", bufs=2) as sb, \
             tc.tile_pool(name="ps", bufs=2, space="PSUM") as ps, \
             tc.tile_pool(name="dram", bufs=1, space="DRAM") as dram:

            # ---------------- constants / persistents
            iota_t = c1.tile([P, P], BF16)
            identb_t = c1.tile([P, P], BF16)
            identf_t = c1.tile([P, P], F32)
            sel_t = c1.tile([P, 2], F32)
            w1_t = c1.tile([P, P], F32)
            w2_t = c1.tile([P, P], F32)
            gamma_t = c1.tile([P, 1], F32)
            beta_t = c1.tile([P, 1], F32)
            rowidx_t = c1.tile([P, 1], F32)
            dinv2_t = c1.tile([P, cfg.blocks], F32)
            nc.sync.dma_start(rowidx_t[:], rowidx_in[:])
            nc.sync.dma_start(dinv2_t[:], dinv2_in[:])
            nc.sync.dma_start(iota_t[:], iota_in[:])
            nc.sync.dma_start(identb_t[:], idb_in[:])
            nc.sync.dma_start(identf_t[:], idf_in[:])
            nc.sync.dma_start(sel_t[:], sel_in[:])
            nc.sync.dma_start(w1_t[:], w1_in[:])
            nc.sync.dma_start(w2_t[:], w2_in[:])
            nc.sync.dma_start(gamma_t[:], gamma_in[:])
            nc.sync.dma_start(beta_t[:], beta_in[:])

            conv1_sb = c1.tile([P, cfg.shard], BF16)     # layer1 conv (pre-BN)
            conv2_sb = c1.tile([P, cfg.shard], F32)      # layer2 conv (pre-BN)

            ag_h_in = dram.tile([cfg.shard, P], BF16)
            ag_h_out = dram.tile([cfg.npad, P], BF16, addr_space="Shared")
            stats_in = [dram.tile([2, P], F32, name=f"stats_in{l}") for l in range(2)]
            stats_out = [dram.tile([2 * cfg.ncores, P], F32, addr_space="Shared",
                                   name=f"stats_out{l}") for l in range(2)]

            slab_allocs = [0]

            def emit_layer(lyr):
                src_ap = x_gsrc if lyr == 0 else ag_h_out
                own_ap = x_own if lyr == 0 else ag_h_in
                w_t = w1_t if lyr == 0 else w2_t
                conv_sb = conv1_sb if lyr == 0 else conv2_sb
                s_part = c1.tile([P, cfg.blocks], F32, name=f"S{lyr}")
                q_part = c1.tile([P, cfg.blocks], F32, name=f"Q{lyr}")

                for bi, pb in enumerate(plan):
                    t0 = pb["tile0"]
                    ntl = pb["ntiles"]
                    slab = sb.tile([P, t_max, P], BF16, tag="slab", bufs=cfg.slab_bufs)
                    if slab_allocs[0] < cfg.slab_bufs:
                        nc.vector.memset(slab[:], 0)
                    slab_allocs[0] += 1
                    idx_t = sb.tile([P, t_max * 8], I16, tag="idx", bufs=2)
                    oh_t = sb.tile([P, t_max * P], BF16, tag="oh_s", bufs=2)
                    nc.sync.dma_start(idx_t[:, :ntl * 8], idx_in[:, t0 * 8:(t0 + ntl) * 8])
                    nc.sync.dma_start(oh_t[:, :ntl * P], oh_in[:, t0 * P:(t0 + ntl) * P])

                    for cl in pb["calls"]:
                        if cl["tiles"] == 0 or cl["reg"] == 0:
                            continue
                        k = cl["bank"]
                        lt0 = cl["tile_off"] - t0
                        nc.gpsimd.dma_gather(
                            slab[:, lt0:lt0 + cl["tiles"], :],
                            src_ap[k * cfg.bank_rows:(k + 1) * cfg.bank_rows, :],
                            idx_t[:, lt0 * 8:(lt0 + cl["tiles"]) * 8],
                            cl["slots"],
                            cl["reg"],
                            P,
                            elem_step=P,
                        )

                    for b in pb["blocks"]:
                        tiles = pb["block_tiles"][b]
                        agg_ps = ps.tile([P, P], F32, tag="agg", bufs=2)
                        for j, t in enumerate(tiles):
                            lt = t - t0
                            nc.tensor.matmul(
                                out=agg_ps[:], lhsT=slab[:, lt, :],
                                rhs=oh_t[:, lt * P:(lt + 1) * P],
                                start=(j == 0), stop=False,
                            )
                        # self-loop: agg[ch, d] += x_own[d, ch] * dinv2[d]
                        oh_s = sb.tile([P, P], BF16, tag="oh", bufs=4)
                        nc.vector.tensor_scalar(
                            oh_s[:], iota_t[:],
                            rowidx_t[:], dinv2_t[:, b:b + 1],
                            OP.is_equal, OP.mult,
                        )
                        xo = sb.tile([P, P], BF16, tag="xo", bufs=3)
                        nc.sync.dma_start(xo[:], own_ap[b * P:(b + 1) * P, :])
                        nc.tensor.matmul(
                            out=agg_ps[:], lhsT=xo[:], rhs=oh_s[:],
                            start=(len(tiles) == 0), stop=True,
                        )
                        aggT = sb.tile([P, P], F32, tag="aggT", bufs=3)
                        nc.vector.tensor_copy(aggT[:], agg_ps[:])
                        cps = ps.tile([P, P], F32, tag="conv", bufs=2)
                        nc.tensor.matmul(out=cps[:], lhsT=w_t[:], rhs=aggT[:],
                                         start=True, stop=True)
                        # copy to conv store + per-channel sum via accumulator
                        nc.scalar.activation(
                            out=conv_sb[:, b * P:(b + 1) * P], in_=cps[:],
                            func=AF.Copy, accum_out=s_part[:, b:b + 1])
                        sq = sb.tile([P, P], F32, tag="sq", bufs=2)
                        nc.scalar.activation(
                            out=sq[:], in_=cps[:], func=AF.Square,
                            accum_out=q_part[:, b:b + 1])

                # ---- stats allgather + affine coefficients
                s_red = sb.tile([P, 1], F32, tag="sred", bufs=2)
                q_red = sb.tile([P, 1], F32, tag="qred", bufs=2)
                nc.vector.tensor_reduce(s_red[:], s_part[:], mybir.AxisListType.X, OP.add)
                nc.vector.tensor_reduce(q_red[:], q_part[:], mybir.AxisListType.X, OP.add)
                nc.sync.dma_start(stats_in[lyr][0], s_red[:])
                nc.sync.dma_start(stats_in[lyr][1], q_red[:])
                nc.gpsimd.collective_compute(
                    "AllGather", OP.bypass,
                    replica_groups=[list(range(cfg.ncores))],
                    ins=[stats_in[lyr].opt()],
                    outs=[stats_out[lyr].opt()],
                )
                stats_sb = sb.tile([P, P], F32, tag="stats_sb", bufs=2)
                nc.vector.memset(stats_sb[:], 0)
                nc.sync.dma_start(stats_sb[:2 * cfg.ncores, :P], stats_out[lyr][:])
                stat_ps = ps.tile([P, 2], F32, tag="stat_ps", bufs=1)
                nc.tensor.matmul(out=stat_ps[:], lhsT=stats_sb[:], rhs=sel_t[:],
                                 start=True, stop=True)
                mu = sb.tile([P, 1], F32, tag="mu", bufs=2)
                msq = sb.tile([P, 1], F32, tag="msq", bufs=2)
                var = sb.tile([P, 1], F32, tag="var", bufs=2)
                sd = sb.tile([P, 1], F32, tag="sd", bufs=2)
                rs = sb.tile([P, 1], F32, tag="rs", bufs=2)
                s_co = sb.tile([P, 1], F32, tag="s_co", bufs=2)
                t_co = sb.tile([P, 1], F32, tag="t_co", bufs=2)
                nc.vector.tensor_scalar(mu[:], stat_ps[:, 0:1], inv_n, None, OP.mult)
                nc.vector.tensor_scalar(msq[:], stat_ps[:, 1:2], inv_n, None, OP.mult)
                nc.vector.tensor_tensor(out=var[:], in0=mu[:], in1=mu[:], op=OP.mult)
                nc.vector.tensor_tensor(out=var[:], in0=msq[:], in1=var[:], op=OP.subtract)
                nc.vector.tensor_scalar(var[:], var[:], EPS, None, OP.add)
                nc.scalar.activation(out=sd[:], in_=var[:], func=AF.Sqrt)
                nc.vector.reciprocal(rs[:], sd[:])
                nc.vector.tensor_tensor(out=s_co[:], in0=gamma_t[:], in1=rs[:], op=OP.mult)
                nc.vector.tensor_tensor(out=t_co[:], in0=mu[:], in1=s_co[:], op=OP.mult)
                nc.vector.tensor_tensor(out=t_co[:], in0=beta_t[:], in1=t_co[:], op=OP.subtract)
                return s_co, t_co

            # ======== layer 1
            s1, t1 = emit_layer(0)
            for b in range(cfg.blocks):
                hT = sb.tile([P, P], BF16, tag="hT", bufs=3)
                nc.scalar.activation(out=hT[:], in_=conv1_sb[:, b * P:(b + 1) * P],
                                     func=AF.Relu, bias=t1[:], scale=s1[:])
                trp = ps.tile([P, P], BF16, tag="trb", bufs=2)
                nc.tensor.transpose(out=trp[:], in_=hT[:], identity=identb_t[:])
                hrow = sb.tile([P, P], BF16, tag="hrow", bufs=3)
                nc.vector.tensor_copy(hrow[:], trp[:])
                nc.sync.dma_start(ag_h_in[b * P:(b + 1) * P, :], hrow[:])
            nc.gpsimd.collective_compute(
                "AllGather", mybir.AluOpType.bypass,
                replica_groups=[list(range(cfg.ncores))],
                ins=[ag_h_in.opt()],
                outs=[ag_h_out.opt()],
            )

            # ======== layer 2
            s2, t2 = emit_layer(1)
            for b in range(cfg.blocks):
                bn = sb.tile([P, P], F32, tag="bn", bufs=3)
                nc.vector.tensor_scalar(bn[:], conv2_sb[:, b * P:(b + 1) * P],
                                        s2[:], t2[:], OP.mult, OP.add)
                xt = sb.tile([P, P], F32, tag="xt", bufs=3)
                nc.sync.dma_start(xt[:], xT_id[:, b * P:(b + 1) * P])
                bn2 = sb.tile([P, P], F32, tag="bn2", bufs=3)
                nc.vector.tensor_tensor(out=bn2[:], in0=bn[:], in1=xt[:], op=OP.add)
                trf = ps.tile([P, P], F32, tag="trf", bufs=1)
                nc.tensor.transpose(out=trf[:], in_=bn2[:], identity=identf_t[:])
                ot = sb.tile([P, P], F32, tag="ot", bufs=3)
                nc.scalar.activation(out=ot[:], in_=trf[:], func=AF.Relu)
                nc.sync.dma_start(out_d[b * P:(b + 1) * P, :], ot[:])

    nc.compile()
    return nc


# ----------------------------------------------------------------------------
# runner
# ----------------------------------------------------------------------------

_CACHE = {}


def _get_module(cfg: Cfg, edge_key, edge_index):
    key = ("mod", cfg.n_real, cfg.shard, edge_key)
    if key not in _CACHE:
        meta, streams = preprocess(edge_index, cfg)
        nc = build_module(cfg, meta)
        _CACHE[key] = (nc, meta, streams)
    return _CACHE[key]


def _make_in_maps(cfg: Cfg, x, W1, W2, gamma2, beta2, streams):
    n, rp, sh = cfg.n_real, cfg.real_per_shard, cfg.shard
    x = np.asarray(x, dtype=np.float32)
    x_pad = np.zeros((cfg.npad, P), dtype=np.float32)
    for c in range(cfg.ncores):
        x_pad[c * sh:c * sh + rp] = x[c * rp:(c + 1) * rp]
    x_bf = x_pad.astype(BF16_NP)

    iota = np.broadcast_to(np.arange(P, dtype=np.float32), (P, P)).astype(BF16_NP)
    identb = np.eye(P, dtype=np.float32).astype(BF16_NP)
    identf = np.eye(P, dtype=np.float32)
    sel = np.zeros((P, 2), dtype=np.float32)
    sel[0:2 * cfg.ncores:2, 0] = 1.0
    sel[1:2 * cfg.ncores:2, 1] = 1.0

    rowidx = np.arange(P, dtype=np.float32).reshape(P, 1)
    in_maps = []
    for c in range(cfg.ncores):
        xT = np.zeros((P, sh), dtype=np.float32)
        xT[:, :rp] = x[c * rp:(c + 1) * rp].T
        st = streams[c]
        in_maps.append(dict(
            x_gsrc=x_bf, x_own=np.ascontiguousarray(x_bf[c * sh:(c + 1) * sh]),
            rowidx=rowidx, dinv2_s=st["dinv2"], xT_id=xT,
            idx_s=st["idx"], oh_s=st["oh"],
            w1=np.asarray(W1, np.float32), w2=np.asarray(W2, np.float32),
            gamma=np.asarray(gamma2, np.float32), beta=np.asarray(beta2, np.float32),
            iota_c=np.asarray(iota), ident_b=np.asarray(identb),
            ident_f=identf, ones_sel=sel,
        ))
    return in_maps


def run(x, W1, b1, W2, b2, gamma2, beta2, edge_index, cfg=CFG_FULL, trace=False):
    from concourse import bass_utils
    ei = np.asarray(edge_index)
    edge_key = hash(ei.tobytes())
    nc, meta, streams = _get_module(cfg, edge_key, ei)
    in_maps = _make_in_maps(cfg, x, W1, W2, gamma2, beta2, streams)
    res = bass_utils.run_bass_kernel_spmd(
        nc, in_maps, core_ids=list(range(cfg.ncores)), trace=trace)
    out = np.empty((cfg.n_real, P), dtype=np.float32)
    rp = cfg.real_per_shard
    for c in range(cfg.ncores):
        out[c * rp:(c + 1) * rp] = res.results[c]["out"][:rp]
    return out, res


def kernel(x, W1, b1, W2, b2, gamma2, beta2, edge_index):
    out, _ = run(x, W1, b1, W2, b2, gamma2, beta2, edge_index)
    return out


# revision 32
# speedup vs baseline: 1.1280x; 1.0562x over previous
"""2-layer GCN block (GCNConv -> BN -> ReLU -> GCNConv -> BN -> +residual -> ReLU)
on 8 TRN2 NeuronCores.

Strategy (graph/data parallel, matches the sharding hint):
- Nodes are padded to 100352 = 8*12544 and sharded by contiguous range; core c
  owns rows [c*12544, (c+1)*12544) (= original nodes [c*12500,(c+1)*12500) plus
  44 pad slots). Edges are bucketed by destination owner on the host.
- GCNConv is reassociated via linearity: agg[dst] = sum_e norm_e * x[src_e]
  (self-loops become ordinary edges with norm = 1/deg), then conv = W.T @ agg
  in transposed layout [ch, node]. The bias cancels exactly through
  training-mode BatchNorm and is dropped.
- Per core, edges sorted by (dst block of 128, src bank of 25088). Source rows
  are fetched with dma_gather (int16 bank-relative indices), scattered into
  the dst block via a one-hot matmul on the TensorEngine accumulating in PSUM:
  onehot[e, d] = (iota[d] == dstmod_e) * norm_e   (one fused DVE op / tile)
  aggT[ch, d] += gathered[e, ch].T @ onehot       (one bf16 matmul / tile)
- BN stats (sum / sum-of-squares per channel) ride a tiny AllGather; the h
  shards move between layers with one 25.7MB bf16 AllGather.
- Per-core gather-group sizes are equalized across cores (pad with idx=0,
  norm=0) so all 8 cores run one identical instruction stream.
"""

import math
import os
import sys
import time

import numpy as np

for _p in ("/opt/trn_rl_repo", "/root/.axon_site/_ro/trn_rl_repo"):
    if os.path.isdir(_p) and _p not in sys.path:
        sys.path.append(_p)

import ml_dtypes

BF16_NP = ml_dtypes.bfloat16

P = 128
EPS = 1e-5


class Cfg:
    def __init__(self, n_real=100000, shard_blocks=98, ncores=8, batch_blocks=4,
                 nbanks=4, slab_bufs=2):
        self.n_real = n_real
        self.ncores = ncores
        self.blocks = shard_blocks          # 128-row blocks per core
        self.shard = shard_blocks * P       # rows per core (padded)
        self.npad = self.shard * ncores
        self.real_per_shard = n_real // ncores
        assert n_real % ncores == 0 and self.real_per_shard <= self.shard
        self.nbanks = nbanks
        assert self.npad % nbanks == 0
        self.bank_rows = self.npad // nbanks
        assert self.bank_rows <= 32767
        self.batch_blocks = batch_blocks
        self.nbatches = math.ceil(self.blocks / batch_blocks)
        self.slab_bufs = slab_bufs


CFG_FULL = Cfg()


# ----------------------------------------------------------------------------
# host-side graph preprocessing
# ----------------------------------------------------------------------------

def preprocess(edge_index, cfg: Cfg):
    """Bucket/sort/pad edges; build per-core device streams + a shared plan."""
    n, rp, sh = cfg.n_real, cfg.real_per_shard, cfg.shard
    src = np.asarray(edge_index[0], dtype=np.int64)
    dst = np.asarray(edge_index[1], dtype=np.int64)

    deg = np.bincount(dst, minlength=n).astype(np.float64) + 1.0
    dinv = 1.0 / np.sqrt(deg)
    norm = (dinv[src] * dinv[dst]).astype(np.float32)

    def to_pad(ids):
        return (ids // rp) * sh + (ids % rp)

    gsrc = to_pad(src)
    gdst = to_pad(dst)
    w = norm

    # self-loop weights per core, laid out [128, blocks] (node b*128+p), pads 0
    dinv2 = np.zeros((cfg.ncores, P, cfg.blocks), dtype=np.float32)
    d2 = (dinv * dinv).astype(np.float32)
    for c in range(cfg.ncores):
        v = np.zeros(sh, dtype=np.float32)
        v[:rp] = d2[c * rp:(c + 1) * rp]
        dinv2[c] = v.reshape(cfg.blocks, P).T

    core = gdst // sh
    dst_local = gdst - core * sh
    block = dst_local // P
    dstmod = (dst_local % P).astype(np.float32)
    bank = gsrc // cfg.bank_rows
    src_rel = (gsrc - bank * cfg.bank_rows).astype(np.int16)

    # per (core, block, bank) counts -> equalized counts
    nb = cfg.nbanks
    gkey = (core * cfg.blocks + block) * nb + bank
    counts = np.bincount(gkey, minlength=cfg.ncores * cfg.blocks * nb)
    counts = counts.reshape(cfg.ncores, cfg.blocks, nb)
    valid_eq = counts.max(axis=0)                         # [blocks, nbanks]
    slot_cnt = ((valid_eq + P - 1) // P) * P              # [blocks, nbanks]

    # ---- shared plan ----------------------------------------------------
    batches = [list(range(i, min(i + cfg.batch_blocks, cfg.blocks)))
               for i in range(0, cfg.blocks, cfg.batch_blocks)]
    plan = []
    tile_base = 0
    # group start position (in slots) inside each core stream, per (block, bank)
    grp_start = np.zeros((cfg.blocks, nb), dtype=np.int64)
    stream_pos = 0
    for bl in batches:
        calls = []
        bt0 = tile_base
        block_tiles = {b: [] for b in bl}
        wtiles = 8  # max gather-call size: 8 tiles = 1024 indices
        for k in range(nb):
            seg_t0 = tile_base
            seg_slots = 0
            seg_valid = 0
            for b in bl:
                grp_start[b, k] = stream_pos
                ntk = int(slot_cnt[b, k]) // P
                block_tiles[b].extend(range(tile_base, tile_base + ntk))
                seg_valid += int(valid_eq[b, k]) if b == bl[-1] else int(slot_cnt[b, k])
                seg_slots += int(slot_cnt[b, k])
                tile_base += ntk
                stream_pos += int(slot_cnt[b, k])
            # split this (batch, bank) segment into windows of <= wtiles tiles
            for w0 in range(0, seg_slots // P, wtiles):
                wt = min(wtiles, seg_slots // P - w0)
                wvalid = max(0, min(seg_valid - w0 * P, wt * P))
                if wvalid == 0:
                    continue
                calls.append(dict(slots=wt * P, reg=int(wvalid), bank=k,
                                  tile_off=seg_t0 + w0, tiles=wt))
        plan.append(dict(blocks=bl, calls=calls, tile0=bt0,
                         ntiles=tile_base - bt0,
                         block_tiles={b: block_tiles[b] for b in bl}))
    tot_tiles = tile_base
    tot_slots = tot_tiles * P

    # ---- per-core streams ----------------------------------------------
    # default fill: pads are idx 0 (valid, norm 0) except each call's trailing
    # region after the last group's equalized count, which is -1 (skipped).
    idx_flat0 = np.full(tot_slots, -1, dtype=np.int16)
    for bl in batches:
        for k in range(nb):
            for b in bl:
                s = grp_start[b, k]
                fill = valid_eq[b, k] if b == bl[-1] else slot_cnt[b, k]
                idx_flat0[s:s + fill] = 0
    streams = []
    for c in range(cfg.ncores):
        sel = np.nonzero(core == c)[0]
        bsel = block[sel]
        ksel = bank[sel]
        key = bsel * nb + ksel
        o = np.argsort(key, kind="stable")
        sel = sel[o]
        key = key[o]
        # rank within group
        starts = np.searchsorted(key, np.arange(cfg.blocks * nb))
        rank = np.arange(len(sel)) - starts[key]
        pos = grp_start.reshape(-1)[key] + rank

        idx_flat = idx_flat0.copy()
        nrm_flat = np.zeros(tot_slots, dtype=np.float32)
        dst_flat = np.full(tot_slots, -1.0, dtype=np.float32)
        idx_flat[pos] = src_rel[sel]
        nrm_flat[pos] = w[sel]
        dst_flat[pos] = dstmod[sel]

        # idx wrap: per call, i -> [i%16 (x8 partitions), col0 + i//16]
        idx_w = np.empty((P, tot_slots // 16), dtype=np.int16)
        for pb in plan:
            for cl in pb["calls"]:
                s0 = cl["tile_off"] * P
                ns = cl["slots"]
                if ns == 0:
                    continue
                wseg = idx_flat[s0:s0 + ns].reshape(ns // 16, 16).T  # [16, cols]
                idx_w[:, s0 // 16:(s0 + ns) // 16] = np.tile(wseg, (8, 1))
        # host-built one-hot scatter matrices, bf16: slot i -> row (i), col dst
        oh_flat = np.zeros((tot_slots, P), dtype=np.float32)
        vmask = dst_flat >= 0
        oh_flat[np.nonzero(vmask)[0], dst_flat[vmask].astype(np.int64)] = nrm_flat[vmask]
        oh_w = np.ascontiguousarray(
            oh_flat.reshape(tot_tiles, P, P).transpose(1, 0, 2)
                   .reshape(P, tot_tiles * P)).astype(BF16_NP)
        streams.append(dict(idx=idx_w, oh=oh_w, dinv2=dinv2[c]))

    meta = dict(plan=plan, tot_tiles=tot_tiles,
                t_max=max(pb["ntiles"] for pb in plan))
    return meta, streams


# ----------------------------------------------------------------------------
# device module
# ----------------------------------------------------------------------------

def build_module(cfg: Cfg, meta):
    import concourse.bacc as bacc
    import concourse.tile as tile
    from concourse import mybir

    F32 = mybir.dt.float32
    BF16 = mybir.dt.bfloat16
    I16 = mybir.dt.int16
    AF = mybir.ActivationFunctionType
    OP = mybir.AluOpType

    plan = meta["plan"]
    tot_tiles = meta["tot_tiles"]
    t_max = meta["t_max"]
    nb = cfg.nbanks

    nc = bacc.Bacc("TRN2", target_bir_lowering=False, debug=False,
                   num_devices=cfg.ncores)

    x_gsrc = nc.dram_tensor("x_gsrc", [cfg.npad, P], BF16, kind="ExternalInput").ap()
    x_own = nc.dram_tensor("x_own", [cfg.shard, P], BF16, kind="ExternalInput").ap()
    dinv2_in = nc.dram_tensor("dinv2_s", [P, cfg.blocks], F32, kind="ExternalInput").ap()
    rowidx_in = nc.dram_tensor("rowidx", [P, 1], F32, kind="ExternalInput").ap()
    xT_id = nc.dram_tensor("xT_id", [P, cfg.shard], F32, kind="ExternalInput").ap()
    idx_in = nc.dram_tensor("idx_s", [P, tot_tiles * 8], I16, kind="ExternalInput").ap()
    oh_in = nc.dram_tensor("oh_s", [P, tot_tiles * P], BF16, kind="ExternalInput").ap()
    w1_in = nc.dram_tensor("w1", [P, P], F32, kind="ExternalInput").ap()
    w2_in = nc.dram_tensor("w2", [P, P], F32, kind="ExternalInput").ap()
    gamma_in = nc.dram_tensor("gamma", [P], F32, kind="ExternalInput").ap()
    beta_in = nc.dram_tensor("beta", [P], F32, kind="ExternalInput").ap()
    iota_in = nc.dram_tensor("iota_c", [P, P], BF16, kind="ExternalInput").ap()
    idb_in = nc.dram_tensor("ident_b", [P, P], BF16, kind="ExternalInput").ap()
    idf_in = nc.dram_tensor("ident_f", [P, P], F32, kind="ExternalInput").ap()
    sel_in = nc.dram_tensor("ones_sel", [P, 2], F32, kind="ExternalInput").ap()

    out_d = nc.dram_tensor("out", [cfg.shard, P], F32, kind="ExternalOutput").ap()

    inv_n = 1.0 / float(cfg.n_real)

    with tile.TileContext(nc) as tc:
        with tc.tile_pool(name="c1", bufs=1) as c1, \
             tc.tile_pool(name="sb", bufs=2) as sb, \
             tc.tile_pool(name="ps", bufs=2, space="PSUM") as ps, \
             tc.tile_pool(name="dram", bufs=1, space="DRAM") as dram:

            # ---------------- constants / persistents
            iota_t = c1.tile([P, P], BF16)
            identb_t = c1.tile([P, P], BF16)
            identf_t = c1.tile([P, P], F32)
            sel_t = c1.tile([P, 2], F32)
            w1_t = c1.tile([P, P], F32)
            w2_t = c1.tile([P, P], F32)
            gamma_t = c1.tile([P, 1], F32)
            beta_t = c1.tile([P, 1], F32)
            rowidx_t = c1.tile([P, 1], F32)
            dinv2_t = c1.tile([P, cfg.blocks], F32)
            nc.sync.dma_start(rowidx_t[:], rowidx_in[:])
            nc.sync.dma_start(dinv2_t[:], dinv2_in[:])
            nc.sync.dma_start(iota_t[:], iota_in[:])
            nc.sync.dma_start(identb_t[:], idb_in[:])
            nc.sync.dma_start(identf_t[:], idf_in[:])
            nc.sync.dma_start(sel_t[:], sel_in[:])
            nc.sync.dma_start(w1_t[:], w1_in[:])
            nc.sync.dma_start(w2_t[:], w2_in[:])
            nc.sync.dma_start(gamma_t[:], gamma_in[:])
            nc.sync.dma_start(beta_t[:], beta_in[:])

            conv1_sb = c1.tile([P, cfg.shard], BF16)     # layer1 conv (pre-BN)
            conv2_sb = c1.tile([P, cfg.shard], BF16)     # layer2 conv (pre-BN)

            ag_h_in = dram.tile([cfg.shard, P], BF16)
            ag_h_out = dram.tile([cfg.npad, P], BF16, addr_space="Shared")
            stats_in = [dram.tile([2, P], F32, name=f"stats_in{l}") for l in range(2)]
            stats_out = [dram.tile([2 * cfg.ncores, P], F32, addr_space="Shared",
                                   name=f"stats_out{l}") for l in range(2)]

            slab_allocs = [0]

            def emit_layer(lyr):
                src_ap = x_gsrc if lyr == 0 else ag_h_out
                own_ap = x_own if lyr == 0 else ag_h_in
                w_t = w1_t if lyr == 0 else w2_t
                conv_sb = conv1_sb if lyr == 0 else conv2_sb
                s_part = c1.tile([P, cfg.blocks], F32, name=f"S{lyr}")
                q_part = c1.tile([P, cfg.blocks], F32, name=f"Q{lyr}")

                for bi, pb in enumerate(plan):
                    t0 = pb["tile0"]
                    ntl = pb["ntiles"]
                    slab = sb.tile([P, t_max, P], BF16, tag="slab", bufs=cfg.slab_bufs)
                    if slab_allocs[0] < cfg.slab_bufs:
                        nc.vector.memset(slab[:], 0)
                    slab_allocs[0] += 1
                    idx_t = sb.tile([P, t_max * 8], I16, tag="idx", bufs=2)
                    oh_t = sb.tile([P, t_max * P], BF16, tag="oh_s", bufs=2)
                    nc.sync.dma_start(idx_t[:, :ntl * 8], idx_in[:, t0 * 8:(t0 + ntl) * 8])
                    nc.sync.dma_start(oh_t[:, :ntl * P], oh_in[:, t0 * P:(t0 + ntl) * P])

                    for cl in pb["calls"]:
                        if cl["tiles"] == 0 or cl["reg"] == 0:
                            continue
                        k = cl["bank"]
                        lt0 = cl["tile_off"] - t0
                        nc.gpsimd.dma_gather(
                            slab[:, lt0:lt0 + cl["tiles"], :],
                            src_ap[k * cfg.bank_rows:(k + 1) * cfg.bank_rows, :],
                            idx_t[:, lt0 * 8:(lt0 + cl["tiles"]) * 8],
                            cl["slots"],
                            cl["reg"],
                            P,
                            elem_step=P,
                        )

                    for b in pb["blocks"]:
                        tiles = pb["block_tiles"][b]
                        agg_ps = ps.tile([P, P], F32, tag="agg", bufs=2)
                        for j, t in enumerate(tiles):
                            lt = t - t0
                            nc.tensor.matmul(
                                out=agg_ps[:], lhsT=slab[:, lt, :],
                                rhs=oh_t[:, lt * P:(lt + 1) * P],
                                start=(j == 0), stop=False,
                            )
                        # self-loop: agg[ch, d] += x_own[d, ch] * dinv2[d]
                        oh_s = sb.tile([P, P], BF16, tag="oh", bufs=4)
                        nc.vector.tensor_scalar(
                            oh_s[:], iota_t[:],
                            rowidx_t[:], dinv2_t[:, b:b + 1],
                            OP.is_equal, OP.mult,
                        )
                        xo = sb.tile([P, P], BF16, tag="xo", bufs=3)
                        nc.sync.dma_start(xo[:], own_ap[b * P:(b + 1) * P, :])
                        nc.tensor.matmul(
                            out=agg_ps[:], lhsT=xo[:], rhs=oh_s[:],
                            start=(len(tiles) == 0), stop=True,
                        )
                        aggT = sb.tile([P, P], F32, tag="aggT", bufs=3)
                        nc.vector.tensor_copy(aggT[:], agg_ps[:])
                        cps = ps.tile([P, P], F32, tag="conv", bufs=2)
                        nc.tensor.matmul(out=cps[:], lhsT=w_t[:], rhs=aggT[:],
                                         start=True, stop=True)
                        # copy to conv store + per-channel sum via accumulator
                        nc.scalar.activation(
                            out=conv_sb[:, b * P:(b + 1) * P], in_=cps[:],
                            func=AF.Copy, accum_out=s_part[:, b:b + 1])
                        sq = sb.tile([P, P], F32, tag="sq", bufs=2)
                        nc.scalar.activation(
                            out=sq[:], in_=cps[:], func=AF.Square,
                            accum_out=q_part[:, b:b + 1])

                # ---- stats allgather + affine coefficients
                s_red = sb.tile([P, 1], F32, tag="sred", bufs=2)
                q_red = sb.tile([P, 1], F32, tag="qred", bufs=2)
                nc.vector.tensor_reduce(s_red[:], s_part[:], mybir.AxisListType.X, OP.add)
                nc.vector.tensor_reduce(q_red[:], q_part[:], mybir.AxisListType.X, OP.add)
                nc.sync.dma_start(stats_in[lyr][0], s_red[:])
                nc.sync.dma_start(stats_in[lyr][1], q_red[:])
                nc.gpsimd.collective_compute(
                    "AllGather", OP.bypass,
                    replica_groups=[list(range(cfg.ncores))],
                    ins=[stats_in[lyr].opt()],
                    outs=[stats_out[lyr].opt()],
                )
                stats_sb = sb.tile([P, P], F32, tag="stats_sb", bufs=2)
                nc.vector.memset(stats_sb[:], 0)
                nc.sync.dma_start(stats_sb[:2 * cfg.ncores, :P], stats_out[lyr][:])
                stat_ps = ps.tile([P, 2], F32, tag="stat_ps", bufs=1)
                nc.tensor.matmul(out=stat_ps[:], lhsT=stats_sb[:], rhs=sel_t[:],
                                 start=True, stop=True)
                mu = sb.tile([P, 1], F32, tag="mu", bufs=2)
                msq = sb.tile([P, 1], F32, tag="msq", bufs=2)
                var = sb.tile([P, 1], F32, tag="var", bufs=2)
                sd = sb.tile([P, 1], F32, tag="sd", bufs=2)
                rs = sb.tile([P, 1], F32, tag="rs", bufs=2)
                s_co = sb.tile([P, 1], F32, tag="s_co", bufs=2)
                t_co = sb.tile([P, 1], F32, tag="t_co", bufs=2)
                nc.vector.tensor_scalar(mu[:], stat_ps[:, 0:1], inv_n, None, OP.mult)
                nc.vector.tensor_scalar(msq[:], stat_ps[:, 1:2], inv_n, None, OP.mult)
                nc.vector.tensor_tensor(out=var[:], in0=mu[:], in1=mu[:], op=OP.mult)
                nc.vector.tensor_tensor(out=var[:], in0=msq[:], in1=var[:], op=OP.subtract)
                nc.vector.tensor_scalar(var[:], var[:], EPS, None, OP.add)
                nc.scalar.activation(out=sd[:], in_=var[:], func=AF.Sqrt)
                nc.vector.reciprocal(rs[:], sd[:])
                nc.vector.tensor_tensor(out=s_co[:], in0=gamma_t[:], in1=rs[:], op=OP.mult)
                nc.vector.tensor_tensor(out=t_co[:], in0=mu[:], in1=s_co[:], op=OP.mult)
                nc.vector.tensor_tensor(out=t_co[:], in0=beta_t[:], in1=t_co[:], op=OP.subtract)
                return s_co, t_co

            # ======== layer 1
            s1, t1 = emit_layer(0)
            for b in range(cfg.blocks):
                hT = sb.tile([P, P], BF16, tag="hT", bufs=3)
                nc.scalar.activation(out=hT[:], in_=conv1_sb[:, b * P:(b + 1) * P],
                                     func=AF.Relu, bias=t1[:], scale=s1[:])
                trp = ps.tile([P, P], BF16, tag="trb", bufs=2)
                nc.tensor.transpose(out=trp[:], in_=hT[:], identity=identb_t[:])
                hrow = sb.tile([P, P], BF16, tag="hrow", bufs=3)
                nc.vector.tensor_copy(hrow[:], trp[:])
                nc.sync.dma_start(ag_h_in[b * P:(b + 1) * P, :], hrow[:])
            nc.gpsimd.collective_compute(
                "AllGather", mybir.AluOpType.bypass,
                replica_groups=[list(range(cfg.ncores))],
                ins=[ag_h_in.opt()],
                outs=[ag_h_out.opt()],
            )

            # ======== layer 2
            s2, t2 = emit_layer(1)
            for b in range(cfg.blocks):
                bn = sb.tile([P, P], F32, tag="bn", bufs=3)
                nc.vector.tensor_scalar(bn[:], conv2_sb[:, b * P:(b + 1) * P],
                                        s2[:], t2[:], OP.mult, OP.add)
                xt = sb.tile([P, P], F32, tag="xt", bufs=3)
                nc.sync.dma_start(xt[:], xT_id[:, b * P:(b + 1) * P])
                bn2 = sb.tile([P, P], F32, tag="bn2", bufs=3)
                nc.vector.tensor_tensor(out=bn2[:], in0=bn[:], in1=xt[:], op=OP.add)
                trf = ps.tile([P, P], F32, tag="trf", bufs=1)
                nc.tensor.transpose(out=trf[:], in_=bn2[:], identity=identf_t[:])
                ot = sb.tile([P, P], F32, tag="ot", bufs=3)
                nc.scalar.activation(out=ot[:], in_=trf[:], func=AF.Relu)
                nc.sync.dma_start(out_d[b * P:(b + 1) * P, :], ot[:])

    nc.compile()
    return nc


# ----------------------------------------------------------------------------
# runner
# ----------------------------------------------------------------------------

_CACHE = {}


def _get_module(cfg: Cfg, edge_key, edge_index):
    key = ("mod", cfg.n_real, cfg.shard, edge_key)
    if key not in _CACHE:
        meta, streams = preprocess(edge_index, cfg)
        nc = build_module(cfg, meta)
        _CACHE[key] = (nc, meta, streams)
    return _CACHE[key]


def _make_in_maps(cfg: Cfg, x, W1, W2, gamma2, beta2, streams):
    n, rp, sh = cfg.n_real, cfg.real_per_shard, cfg.shard
    x = np.asarray(x, dtype=np.float32)
    x_pad = np.zeros((cfg.npad, P), dtype=np.float32)
    for c in range(cfg.ncores):
        x_pad[c * sh:c * sh + rp] = x[c * rp:(c + 1) * rp]
    x_bf = x_pad.astype(BF16_NP)

    iota = np.broadcast_to(np.arange(P, dtype=np.float32), (P, P)).astype(BF16_NP)
    identb = np.eye(P, dtype=np.float32).astype(BF16_NP)
    identf = np.eye(P, dtype=np.float32)
    sel = np.zeros((P, 2), dtype=np.float32)
    sel[0:2 * cfg.ncores:2, 0] = 1.0
    sel[1:2 * cfg.ncores:2, 1] = 1.0

    rowidx = np.arange(P, dtype=np.float32).reshape(P, 1)
    in_maps = []
    for c in range(cfg.ncores):
        xT = np.zeros((P, sh), dtype=np.float32)
        xT[:, :rp] = x[c * rp:(c + 1) * rp].T
        st = streams[c]
        in_maps.append(dict(
            x_gsrc=x_bf, x_own=np.ascontiguousarray(x_bf[c * sh:(c + 1) * sh]),
            rowidx=rowidx, dinv2_s=st["dinv2"], xT_id=xT,
            idx_s=st["idx"], oh_s=st["oh"],
            w1=np.asarray(W1, np.float32), w2=np.asarray(W2, np.float32),
            gamma=np.asarray(gamma2, np.float32), beta=np.asarray(beta2, np.float32),
            iota_c=np.asarray(iota), ident_b=np.asarray(identb),
            ident_f=identf, ones_sel=sel,
        ))
    return in_maps


def run(x, W1, b1, W2, b2, gamma2, beta2, edge_index, cfg=CFG_FULL, trace=False):
    from concourse import bass_utils
    ei = np.asarray(edge_index)
    edge_key = hash(ei.tobytes())
    nc, meta, streams = _get_module(cfg, edge_key, ei)
    in_maps = _make_in_maps(cfg, x, W1, W2, gamma2, beta2, streams)
    res = bass_utils.run_bass_kernel_spmd(
        nc, in_maps, core_ids=list(range(cfg.ncores)), trace=trace)
    out = np.empty((cfg.n_real, P), dtype=np.float32)
    rp = cfg.real_per_shard
    for c in range(cfg.ncores):
        out[c * rp:(c + 1) * rp] = res.results[c]["out"][:rp]
    return out, res


def kernel(x, W1, b1, W2, b2, gamma2, beta2, edge_index):
    out, _ = run(x, W1, b1, W2, b2, gamma2, beta2, edge_index)
    return out
